# revision 56
# baseline (speedup 1.0000x reference)
"""Trainium2 Bass kernel for nn_CLCRec contrastive loss (v2: dma_gather).

Strategy (fully local per core, no collectives):
  - Batch rows sharded 8 ways (2048 rows x 17 group entries per core).
  - All table row fetches use SWDGE dma_gather (0.34ns/desc issue) instead of
    per-128-row indirect DMAs (994ns fixed each); this removes the Pool-engine
    SWDGE-issue bottleneck (655us busy in v1).
  - dma_gather indices are int16 (<32768), so gathers are two-level:
      L1: the core's ~32k *unique* item/user ids, sorted (so ids are
          bank-contiguous), gathered bank-by-bank (7 banks span 200000 rows)
          into a dense relabeled slot space.
      Items are encoded (bf16 MLP), assembled into scratch rows
          [f_hat | f | e | pad] (512B) and streamed to an HBM scratch table;
          user rows go to a second f32 scratch table.
      L2: per batch-tile, ONE gather per table fetches the per-entry rows
          from scratch (slot count <= 32768 fits int16) in g-major order so
          the dst lands as [128 batch rows, 17, elems].
  - Phase 2 computes d1 = <p_hat, f_hat> (both pre-normalized) and
    d2 = <u, select(mask, f, e)>, then the standard exp/log loss tail;
    per-core partial sums are reduced on host.
"""

import os
import sys

import numpy as np

for _p in ("/opt/trn_rl_repo", os.path.expanduser("~/.axon_site/_ro/trn_rl_repo")):
    if os.path.isdir(_p) and _p not in sys.path:
        sys.path.insert(0, _p)

import ml_dtypes

import concourse.bacc as bacc
import concourse.mybir as mybir
import concourse.tile as tile
from concourse import bass_utils
from concourse.library_config import mlp as mlp_lib


F32 = mybir.dt.float32
BF16 = mybir.dt.bfloat16
I32 = mybir.dt.int32
I16 = mybir.dt.int16
AF = mybir.ActivationFunctionType
ALU = mybir.AluOpType
AX = mybir.AxisListType

NUM_USER = 200000
NUM_ITEM = 200000
DIM_E = 64
DIM_FEAT = 128
B = 16384
G = 17  # 1 + num_neg
TEMP = 0.2
LR_LAMBDA = 0.5

NCORE = 8
BC = B // NCORE          # 2048 batch rows per core
NT = BC // 128           # 16 batch tiles (128 batch rows each)
EPT = 128 * G            # 2176 entries per batch tile
ICOL = EPT // 16         # 136 idx cols per batch tile

# 7 gather banks; int16 gather indices reach < 32768 rows past each base.
BANK = 32768
NBANK = 7

_CACHE: dict = {}

_NEEDED_AF = None


def _patch_act_tables():
    """Force every activation we emit to resolve to the combined
    natural_log_exp_and_others set so no per-anchor table swapping occurs."""
    global _NEEDED_AF
    if _CACHE.get("act_patched"):
        return
    _NEEDED_AF = {AF.Ln, AF.Exp, AF.Prelu, AF.Copy, AF.Identity}
    import concourse.hw_specs as hw_specs
    orig = hw_specs.get_activation_tables

    def patched(module_arch):
        tabs = orig(module_arch)
        out = {}
        for name, fns in tabs.items():
            if name == "natural_log_exp_and_others":
                out[name] = fns
            else:
                out[name] = fns - _NEEDED_AF
        return out

    bacc.get_activation_tables = patched
    _CACHE["act_patched"] = True


def _build(PI, BI, PU, BU):
    """PI / PU: per-bank padded slot counts (multiples of 128); BI / BU: bank
    base row ids. Shared by all cores; sum(PI), sum(PU) <= 32768."""
    _patch_act_tables()
    NBI = sum(PI) // 128          # item slot blocks
    NBI4 = -(-NBI // 4) * 4       # padded to whole encoder chunks
    NBU = sum(PU) // 128          # user slot blocks
    NCH = NBI4 // 4
    assert NBI4 <= 256 and NBU <= 256

    STAGE = int(os.environ.get("KERNEL_STAGE", "3"))
    nc = bacc.Bacc("TRN2", target_bir_lowering=False, debug=False,
                   num_devices=NCORE, num_swdge_queues=4)

    evb_d = nc.dram_tensor("evb", [NUM_ITEM, 256], BF16, kind="ExternalInput")
    idu_d = nc.dram_tensor("idu", [NUM_USER, DIM_E], F32, kind="ExternalInput")
    w1_d = nc.dram_tensor("w1", [DIM_FEAT, 256], BF16, kind="ExternalInput")
    b1_d = nc.dram_tensor("b1", [256], F32, kind="ExternalInput")
    w2_d = nc.dram_tensor("w2", [128, 128], BF16, kind="ExternalInput")
    b2_d = nc.dram_tensor("b2", [DIM_E], F32, kind="ExternalInput")
    i1i_d = nc.dram_tensor("i1i", [128, sum(PI) // 16], I16, kind="ExternalInput")
    i1u_d = nc.dram_tensor("i1u", [128, sum(PU) // 16], I16, kind="ExternalInput")
    i2i_d = nc.dram_tensor("i2i", [128, NT * ICOL], I16, kind="ExternalInput")
    i2u_d = nc.dram_tensor("i2u", [128, NT * ICOL], I16, kind="ExternalInput")
    mk_d = nc.dram_tensor("mask", [NT, 128, G], F32, kind="ExternalInput")
    id_d = nc.dram_tensor("ident", [128, 128], F32, kind="ExternalInput")
    out_d = nc.dram_tensor("acc_out", [128, 2], F32, kind="ExternalOutput")

    scri_d = nc.dram_tensor("scri", [128, NBI4, 256], BF16, kind="Internal")
    scru_d = nc.dram_tensor("scru", [128, NBU, DIM_E], F32, kind="Internal")
    stgu_d = nc.dram_tensor("stgu", [128, NT * G, DIM_E], F32, kind="Internal")

    with tile.TileContext(nc) as tc:
        nc.gpsimd.load_library(mlp_lib)
        with tc.tile_pool(name="const", bufs=1) as cp:
            ident = cp.tile([128, 128], F32, tag="ident")
            nc.sync.dma_start(out=ident[:], in_=id_d[:])

            w1sb = cp.tile([128, 256], BF16, tag="w1sb")
            nc.sync.dma_start(out=w1sb[:], in_=w1_d[:])
            w2sb = cp.tile([128, 128], BF16, tag="w2sb")
            nc.sync.dma_start(out=w2sb[:], in_=w2_d[:])
            b1sb = cp.tile([128, 2], F32, tag="b1sb")
            nc.sync.dma_start(out=b1sb[:], in_=b1_d[:].rearrange("(h p) -> p h", p=128))
            b2sb = cp.tile([64, 1], F32, tag="b2sb")
            nc.sync.dma_start(out=b2sb[:], in_=b2_d[:].rearrange("(e o) -> e o", o=1))
            acc = cp.tile([128, 2], F32, tag="acc")
            nc.vector.memset(acc[:], 0.0)

            # single_packet gathers cap at 1024 idxs per call; round-robin the
            # 4 SWDGE queues (each runs on its own Q7 cpu pair -> parallel
            # descriptor generation).
            qrr = [0]

            def gather_split(dst3, blk0, src, idxs, coff0, total, elem):
                done = 0
                while done < total:
                    n = min(1024, total - done)
                    nc.gpsimd.dma_gather(
                        dst3[:, blk0 + done // 128:blk0 + (done + n) // 128, :],
                        src,
                        idxs[:, coff0 + done // 16:coff0 + (done + n) // 16],
                        n, n, elem, queue_num=qrr[0] % 4)
                    qrr[0] += 1
                    done += n

            # ---------------- phase I: items (gather uniques + encode) ------
            with tc.tile_pool(name="l1i", bufs=1) as ip, \
                 tc.tile_pool(name="enc", bufs=3) as ep, \
                 tc.tile_pool(name="l1u", bufs=2) as up, \
                 tc.tile_pool(name="psT", bufs=2, space="PSUM") as psT, \
                 tc.tile_pool(name="psO", bufs=2, space="PSUM") as psO, \
                 tc.tile_pool(name="psH", bufs=2, space="PSUM") as psH, \
                 tc.tile_pool(name="psF", bufs=2, space="PSUM") as psF:

                i1i = ip.tile([128, sum(PI) // 16], I16, tag="i1i")
                nc.sync.dma_start(out=i1i[:], in_=i1i_d[:])
                vdst = ip.tile([128, NBI4 * 256], BF16, tag="vdst")
                vdst3 = vdst[:].rearrange("p (b e) -> p b e", e=256)
                if NBI4 > NBI:
                    nc.vector.memset(vdst3[:, NBI:NBI4, :], 0.0)
                off = 0
                coff = 0
                for b in range(NBANK):
                    gather_split(
                        vdst3, off,
                        evb_d[BI[b]:min(BI[b] + BANK, NUM_ITEM), :],
                        i1i, coff, PI[b], 256)
                    off += PI[b] // 128
                    coff += PI[b] // 16

                # user L1 + per-entry user stream both run during the encoder
                # window (the Q7 queues are otherwise idle then); the per-entry
                # user rows are staged to DRAM and dense-read in phase III.
                i1u = up.tile([128, sum(PU) // 16], I16, tag="i1u", bufs=1)
                nc.sync.dma_start(out=i1u[:], in_=i1u_d[:])
                i2u = up.tile([128, NT * ICOL], I16, tag="i2u", bufs=1)
                nc.sync.dma_start(out=i2u[:], in_=i2u_d[:])
                # split banks into ~8 groups to keep the staging tile small
                groups = []
                cur = []
                cum = 0
                for b in range(NBANK):
                    cur.append(b)
                    cum += PU[b] // 128
                    if cum >= NBU // 8:
                        groups.append(cur)
                        cur = []
                        cum = 0
                if cur:
                    groups.append(cur)
                umax = max(sum(PU[b] // 128 for b in g) for g in groups)
                uoff = 0
                ucoff = 0
                for banks in groups:
                    blocks = sum(PU[b] // 128 for b in banks)
                    udst = up.tile([128, umax * DIM_E], F32, tag="udst", bufs=1)
                    ud3 = udst[:].rearrange("p (b e) -> p b e", e=DIM_E)
                    hoff = 0
                    for b in banks:
                        gather_split(
                            ud3, hoff,
                            idu_d[BU[b]:min(BU[b] + BANK, NUM_USER), :],
                            i1u, ucoff, PU[b], DIM_E)
                        hoff += PU[b] // 128
                        ucoff += PU[b] // 16
                    nc.sync.dma_start(
                        out=scru_d[:, uoff:uoff + blocks, :],
                        in_=ud3[:, 0:blocks, :])
                    uoff += blocks
                rows_u = scru_d[:].rearrange("p b e -> (p b) e")
                for bp in range(NT // 2):
                    stg = up.tile([128, 2 * G * DIM_E], F32, tag="stg", bufs=2)
                    stg3 = stg[:].rearrange("p (g e) -> p g e", e=DIM_E)
                    gather_split(stg3, 0, rows_u, i2u, 2 * bp * ICOL,
                                 2 * EPT, DIM_E)
                    nc.sync.dma_start(
                        out=stgu_d[:, 2 * bp * G:(2 * bp + 2) * G, :],
                        in_=stg3)

                def enc_chunk(q):
                    v3 = vdst3[:, 4 * q:4 * q + 4, 64:192]   # [128,4,128] bf16
                    sq = ep.tile([128, 512], F32, tag="sq")
                    sq3 = sq[:].rearrange("p (j e) -> p j e", e=DIM_FEAT)
                    ss = ep.tile([128, 4], F32, tag="ss")
                    nc.vector.tensor_tensor(out=sq3, in0=v3, in1=v3, op=ALU.mult)
                    nc.vector.tensor_reduce(out=ss[:], in_=sq3, op=ALU.add, axis=AX.X)
                    nc.scalar.activation(out=ss[:], in_=ss[:], func=AF.Ln)
                    nc.scalar.activation(out=ss[:], in_=ss[:], func=AF.Exp, scale=-0.5)
                    vn = ep.tile([128, 512], F32, tag="vn")
                    vn3 = vn[:].rearrange("p (j e) -> p j e", e=DIM_FEAT)
                    nc.vector.tensor_tensor(
                        out=vn3, in0=v3,
                        in1=ss[:].rearrange("p (j o) -> p j o", o=1)
                            .to_broadcast([128, 4, DIM_FEAT]),
                        op=ALU.mult)
                    vT_ps = psT.tile([128, 512], F32, tag="tp")
                    for j in range(4):
                        nc.tensor.transpose(
                            out=vT_ps[:, j * 128:(j + 1) * 128],
                            in_=vn[:, j * 128:(j + 1) * 128],
                            identity=ident[:])
                    vT = ep.tile([128, 512], BF16, tag="vT")
                    nc.scalar.copy(out=vT[:], in_=vT_ps[:])
                    h_sb = ep.tile([128, 1024], BF16, tag="hsb")
                    for h in range(2):
                        h_ps = psH.tile([128, 512], F32, tag="hp")
                        nc.tensor.matmul(
                            out=h_ps[:], lhsT=w1sb[:, h * 128:(h + 1) * 128],
                            rhs=vT[:], start=True, stop=True)
                        nc.scalar.activation(
                            out=h_sb[:, h * 512:(h + 1) * 512], in_=h_ps[:],
                            func=AF.Prelu, bias=b1sb[:, h:h + 1], scale=1.0,
                            alpha=0.01)
                    f_ps = psF.tile([64, 512], F32, tag="fp")
                    nc.tensor.matmul(out=f_ps[:], lhsT=w2sb[:, 0:64],
                                     rhs=h_sb[:, 0:512], start=True, stop=False)
                    nc.tensor.matmul(out=f_ps[:], lhsT=w2sb[:, 64:128],
                                     rhs=h_sb[:, 512:1024], start=False, stop=True)
                    f_sb = ep.tile([64, 512], F32, tag="fsb")
                    nc.scalar.activation(out=f_sb[:], in_=f_ps[:], func=AF.Identity,
                                         bias=b2sb[:, 0:1])
                    Fo_ps = psO.tile([128, 256], F32, tag="fo")
                    for j in range(4):
                        nc.tensor.transpose(
                            out=Fo_ps[:, j * 64:(j + 1) * 64],
                            in_=f_sb[:, j * 128:(j + 1) * 128],
                            identity=ident[:64, :64])
                    Fo3 = Fo_ps[:].rearrange("p (j e) -> p j e", e=64)
                    sc = ep.tile([128, 4 * 256], BF16, tag="sc", bufs=3)
                    sc3 = sc[:].rearrange("p (j e) -> p j e", e=256)
                    nc.vector.tensor_copy(out=sc3[:, :, 64:128], in_=Fo3)
                    fsq = ep.tile([128, 256], F32, tag="fsq")
                    fsq3 = fsq[:].rearrange("p (j e) -> p j e", e=64)
                    fn = ep.tile([128, 4], F32, tag="fn")
                    nc.vector.tensor_tensor(out=fsq3, in0=sc3[:, :, 64:128],
                                            in1=sc3[:, :, 64:128], op=ALU.mult)
                    nc.vector.tensor_reduce(out=fn[:], in_=fsq3, op=ALU.add, axis=AX.X)
                    nc.scalar.activation(out=fn[:], in_=fn[:], func=AF.Ln)
                    nc.scalar.activation(out=fn[:], in_=fn[:], func=AF.Exp, scale=-0.5)
                    nc.vector.tensor_tensor(
                        out=sc3[:, :, 0:64], in0=Fo3,
                        in1=fn[:].rearrange("p (j o) -> p j o", o=1)
                            .to_broadcast([128, 4, 64]),
                        op=ALU.mult)
                    nc.scalar.activation(out=sc3[:, :, 128:192],
                                         in_=vdst3[:, 4 * q:4 * q + 4, 0:64],
                                         func=AF.Copy)
                    nc.sync.dma_start(out=scri_d[:, 4 * q:4 * q + 4, :], in_=sc3)

                for q in range(NCH if STAGE >= 1 else 0):
                    enc_chunk(q)

            # ---------------- phase III: per-entry streams + loss ----------
            with tc.tile_pool(name="ph2", bufs=3) as pp, \
                 tc.tile_pool(name="prd", bufs=3) as prp:
                i2i = pp.tile([128, NT * ICOL], I16, tag="i2i", bufs=1)
                nc.sync.dma_start(out=i2i[:], in_=i2i_d[:])
                rows_i = scri_d[:].rearrange("p b e -> (p b) e")

                def ph2_super(sp):
                    """Gather four batch-tiles' item rows in one split-gather
                    (8704 idxs = 8x1024 + 512 -> 9 calls); user rows dense-read
                    from the DRAM stage written during the encoder window."""
                    ti_p = pp.tile([128, 4 * G * 256], BF16, tag="ti", bufs=2)
                    ti_p3 = ti_p[:].rearrange("p (g e) -> p g e", e=256)
                    gather_split(ti_p3, 0, rows_i, i2i, 4 * sp * ICOL,
                                 4 * EPT, 256)
                    tu_p = pp.tile([128, 4 * G * 64], F32, tag="tu", bufs=2)
                    nc.sync.dma_start(
                        out=tu_p[:].rearrange("p (g e) -> p g e", e=DIM_E),
                        in_=stgu_d[:, 4 * sp * G:(4 * sp + 4) * G, :])
                    for q in range(4):
                        ph2_tile(4 * sp + q,
                                 ti_p[:, q * G * 256:(q + 1) * G * 256],
                                 tu_p[:, q * G * 64:(q + 1) * G * 64])

                def ph2_tile(bt, ti, tu):
                    ti3 = ti.rearrange("p (g e) -> p g e", e=256)
                    tu3 = tu.rearrange("p (g e) -> p g e", e=64)
                    mk = pp.tile([128, G], F32, tag="mk")
                    nc.sync.dma_start(out=mk[:], in_=mk_d[bt])

                    Fh3 = ti3[:, :, 0:64]
                    Fr3 = ti3[:, :, 64:128]
                    E3 = ti3[:, :, 128:192]

                    # p_hat from the positive item's raw embedding (g=0 slot)
                    E0 = ti[:, 128:192]
                    prP = pp.tile([128, 64], F32, tag="prP")
                    dPP = pp.tile([128, 1], F32, tag="dPP")
                    nc.vector.tensor_tensor(out=prP[:], in0=E0, in1=E0, op=ALU.mult)
                    nc.vector.tensor_reduce(
                        out=dPP[:], in_=prP[:].rearrange("p (o e) -> p o e", o=1),
                        op=ALU.add, axis=AX.X)
                    nc.scalar.activation(out=dPP[:], in_=dPP[:], func=AF.Ln)
                    nc.scalar.activation(out=dPP[:], in_=dPP[:], func=AF.Exp,
                                         scale=-0.5)
                    ph = pp.tile([128, 64], BF16, tag="ph")
                    nc.vector.tensor_tensor(
                        out=ph[:].rearrange("p (o e) -> p o e", o=1),
                        in0=E0.rearrange("p (o e) -> p o e", o=1),
                        in1=dPP[:].rearrange("p (o e) -> p o e", e=1)
                            .to_broadcast([128, 1, 64]),
                        op=ALU.mult)
                    ph3b = ph[:].rearrange("p (g e) -> p g e", g=1) \
                        .to_broadcast([128, G, 64])

                    prA = prp.tile([128, G * 64], F32, tag="prA")
                    prA3 = prA[:].rearrange("p (g e) -> p g e", e=64)
                    d1 = pp.tile([128, G], F32, tag="d1")
                    nc.vector.tensor_tensor(out=prA3, in0=Fh3, in1=ph3b, op=ALU.mult)
                    nc.vector.tensor_reduce(out=d1[:], in_=prA3, op=ALU.add, axis=AX.X)

                    # d2 = <u, e> + m * (<u, f> - <u, e>)  (scalar select)
                    prC = prp.tile([128, G * 64], F32, tag="prC")
                    prC3 = prC[:].rearrange("p (g e) -> p g e", e=64)
                    dA = pp.tile([128, G], F32, tag="dA")
                    nc.vector.tensor_tensor(out=prC3, in0=tu3, in1=E3, op=ALU.mult)
                    nc.vector.tensor_reduce(out=dA[:], in_=prC3, op=ALU.add, axis=AX.X)
                    prB = prp.tile([128, G * 64], F32, tag="prB")
                    prB3 = prB[:].rearrange("p (g e) -> p g e", e=64)
                    dB = pp.tile([128, G], F32, tag="dB")
                    nc.vector.tensor_tensor(out=prB3, in0=tu3, in1=Fr3, op=ALU.mult)
                    nc.vector.tensor_reduce(out=dB[:], in_=prB3, op=ALU.add, axis=AX.X)
                    d2 = pp.tile([128, G], F32, tag="d2")
                    nc.vector.tensor_tensor(out=d2[:], in0=dB[:], in1=dA[:],
                                            op=ALU.subtract)
                    nc.vector.tensor_tensor(out=d2[:], in0=d2[:], in1=mk[:],
                                            op=ALU.mult)
                    nc.vector.tensor_tensor(out=d2[:], in0=d2[:], in1=dA[:],
                                            op=ALU.add)

                    # loss tail: -log(pos/tot) = ln(tot) - d[:,0]/T
                    s1 = pp.tile([128, G], F32, tag="s1")
                    s2 = pp.tile([128, G], F32, tag="s2")
                    tot = pp.tile([128, 2], F32, tag="tot")
                    nc.scalar.activation(out=s1[:], in_=d1[:], func=AF.Exp,
                                         scale=1.0 / TEMP, accum_out=tot[:, 0:1])
                    nc.scalar.activation(out=s2[:], in_=d2[:], func=AF.Exp,
                                         scale=1.0 / TEMP, accum_out=tot[:, 1:2])
                    nc.scalar.activation(out=tot[:], in_=tot[:], func=AF.Ln)
                    dd = pp.tile([128, 2], F32, tag="dd")
                    nc.scalar.activation(out=dd[:, 0:1], in_=d1[:, 0:1],
                                         func=AF.Copy, scale=-1.0 / TEMP)
                    nc.scalar.activation(out=dd[:, 1:2], in_=d2[:, 0:1],
                                         func=AF.Copy, scale=-1.0 / TEMP)
                    nc.vector.tensor_tensor(out=acc[:], in0=acc[:], in1=tot[:],
                                            op=ALU.add)
                    nc.vector.tensor_tensor(out=acc[:], in0=acc[:], in1=dd[:],
                                            op=ALU.add)

                for sp in range(NT // 4 if STAGE >= 3 else 0):
                    ph2_super(sp)

            nc.sync.dma_start(out=out_d[:], in_=acc[:])

    nc.compile()
    return nc


def _wrap_idx(idx):
    """idx list -> [128, ceil(n/16)] int16 wrapped (idx i at [i%16, i//16]),
    replicated across the 8 groups of 16 partitions."""
    idx = np.asarray(idx, np.int16)
    n = len(idx)
    cols = -(-n // 16)
    if cols * 16 > n:
        fill = idx[-1] if n else np.int16(0)
        idx = np.concatenate([idx, np.full(cols * 16 - n, fill, np.int16)])
    w = np.ascontiguousarray(idx.reshape(cols, 16).T)
    return np.tile(w, (8, 1))


def _bank_plan(uniqs):
    """Position-based banking: slot space = ceil(max_core_uniques/128)*128
    slots split into 7 contiguous bank segments; bank b of core c takes the
    core's sorted unique ids at positions [C_b, C_b + n) (n <= P_b). Bank base
    = min over cores of the id at position C_b, so every relative index is in
    [0, 32768)."""
    smax = max(len(u) for u in uniqs)
    nbt = -(-smax // 128)
    k, r = divmod(nbt, NBANK)
    blocks = [k + 1] * r + [k] * (NBANK - r)
    P = [bl * 128 for bl in blocks]
    C = np.concatenate([[0], np.cumsum(P)])
    bases = []
    for b in range(NBANK):
        lo = min(int(u[C[b]]) for u in uniqs if C[b] < len(u))
        bases.append(lo)
        for u in uniqs:
            end = min(C[b + 1], len(u))
            if end > C[b]:
                assert int(u[end - 1]) - lo < BANK, "bank window overflow"
    return P, bases


def _host_prep(user_tensor, item_tensor, rand_index):
    it = np.clip(item_tensor.astype(np.int64) - NUM_USER, 0, NUM_ITEM - 1)
    ut = user_tensor.astype(np.int64)
    mask = np.zeros(B * G, np.float32)
    mask[np.asarray(rand_index, dtype=np.int64)] = 1.0
    mask = mask.reshape(B, G)

    cores = []
    for k in range(NCORE):
        sl = slice(k * BC, (k + 1) * BC)
        itc, utc = it[sl], ut[sl]
        cores.append((itc, utc, mask[sl], np.unique(itc), np.unique(utc)))

    PI, BI = _bank_plan([c[3] for c in cores])
    PU, BU = _bank_plan([c[4] for c in cores])
    assert sum(PI) <= BANK and sum(PU) <= BANK, (PI, PU)
    NBI4 = -(-(sum(PI) // 128) // 4) * 4
    NBU = sum(PU) // 128

    def idx_arrays(u, P, bases, nb_total):
        """Returns (l1_idx_wrapped, j_of_unique) for one core."""
        C = np.concatenate([[0], np.cumsum(P)])
        l1 = np.zeros(sum(P), np.int16)
        slots = np.zeros(len(u), np.int64)
        for b in range(NBANK):
            pos, end = int(C[b]), min(int(C[b + 1]), len(u))
            n = max(end - pos, 0)
            ids = u[pos:end]
            rel = (ids - bases[b]).astype(np.int64)
            assert n == 0 or (rel.min() >= 0 and rel.max() < BANK)
            l1[pos:pos + n] = rel.astype(np.int16)
            if n < P[b]:
                l1[pos + n:C[b + 1]] = l1[pos + n - 1] if n else 0
            slots[pos:end] = pos + np.arange(n)
        j = (slots % 128) * nb_total + slots // 128
        assert len(j) == 0 or j.max() < 32768
        return _wrap_idx(l1), j.astype(np.int16)

    per_core = []
    for (itc, utc, mkc, ui, uu) in cores:
        l1i, ji = idx_arrays(ui, PI, BI, NBI4)
        l1u, ju = idx_arrays(uu, PU, BU, NBU)
        # L2 idx in g-major order per batch tile: i = g*128 + r
        je = ji[np.searchsorted(ui, itc)]       # [2048, 17]
        jue = ju[np.searchsorted(uu, utc)]
        l2i = je.reshape(NT, 128, G).transpose(0, 2, 1).reshape(NT, EPT)
        l2u = jue.reshape(NT, 128, G).transpose(0, 2, 1).reshape(NT, EPT)

        def wrap_l2(l2):
            out = np.zeros((16, NT * ICOL), np.int16)
            for bt in range(NT):
                out[:, bt * ICOL:(bt + 1) * ICOL] = \
                    l2[bt].reshape(ICOL, 16).T
            return np.tile(out, (8, 1))

        per_core.append({
            "i1i": l1i, "i1u": l1u,
            "i2i": wrap_l2(l2i), "i2u": wrap_l2(l2u),
            "mask": np.ascontiguousarray(mkc.reshape(NT, 128, G)),
        })
    return PI, BI, PU, BU, per_core


def kernel(v_feat, id_embedding, W1, b1, W2, b2, user_tensor, item_tensor,
           rand_index):
    v_feat = np.asarray(v_feat, dtype=np.float32)
    id_embedding = np.asarray(id_embedding, dtype=np.float32)
    # evb row: [e(0:64) | v(64:192) | 0(192:256)] bf16
    evb = np.zeros((NUM_ITEM, 256), ml_dtypes.bfloat16)
    evb[:, 0:64] = id_embedding[NUM_USER:]
    evb[:, 64:192] = v_feat
    idu = np.ascontiguousarray(id_embedding[:NUM_USER])
    W1b = np.ascontiguousarray(W1, dtype=np.float32).astype(ml_dtypes.bfloat16)
    b1f = np.ascontiguousarray(b1, dtype=np.float32)
    W2f = np.ascontiguousarray(W2, dtype=np.float32)
    # pack W2 halves side by side: [128, 0:64] = W2[0:128], [128, 64:128] = W2[128:256]
    W2b = np.concatenate([W2f[0:128, :], W2f[128:256, :]], axis=1) \
        .astype(ml_dtypes.bfloat16)
    b2f = np.ascontiguousarray(b2, dtype=np.float32)

    PI, BI, PU, BU, per_core = _host_prep(user_tensor, item_tensor, rand_index)

    key = (tuple(PI), tuple(BI), tuple(PU), tuple(BU))
    if _CACHE.get("key") != key:
        _CACHE["nc"] = _build(PI, BI, PU, BU)
        _CACHE["key"] = key
    nc = _CACHE["nc"]

    in_maps = []
    for k in range(NCORE):
        m = {"evb": evb, "idu": idu, "w1": W1b, "b1": b1f, "w2": W2b,
             "b2": b2f, "ident": np.eye(128, dtype=np.float32)}
        m.update(per_core[k])
        in_maps.append(m)
    trace = bool(int(os.environ.get("KERNEL_TRACE", "0")))
    res = bass_utils.run_bass_kernel_spmd(
        nc, in_maps, core_ids=list(range(NCORE)), trace=trace)
    _CACHE["last_results"] = res
    accs = np.stack([r["acc_out"] for r in res.results])  # [8, 128, 2]
    sums = accs.sum(axis=(0, 1), dtype=np.float64)
    l1 = sums[0] / B
    l2 = sums[1] / B
    return np.array(LR_LAMBDA * l1 + (1.0 - LR_LAMBDA) * l2, dtype=np.float32)


# revision 59
# speedup vs baseline: 1.2691x; 1.2691x over previous
"""Trainium2 Bass kernel for nn_CLCRec contrastive loss (v2: dma_gather).

Strategy (fully local per core, no collectives):
  - Batch rows sharded 8 ways (2048 rows x 17 group entries per core).
  - All table row fetches use SWDGE dma_gather (0.34ns/desc issue) instead of
    per-128-row indirect DMAs (994ns fixed each); this removes the Pool-engine
    SWDGE-issue bottleneck (655us busy in v1).
  - dma_gather indices are int16 (<32768), so gathers are two-level:
      L1: the core's ~32k *unique* item/user ids, sorted (so ids are
          bank-contiguous), gathered bank-by-bank (7 banks span 200000 rows)
          into a dense relabeled slot space.
      Items are encoded (bf16 MLP), assembled into scratch rows
          [f_hat | f | e | pad] (512B) and streamed to an HBM scratch table;
          user rows go to a second f32 scratch table.
      L2: per batch-tile, ONE gather per table fetches the per-entry rows
          from scratch (slot count <= 32768 fits int16) in g-major order so
          the dst lands as [128 batch rows, 17, elems].
  - Phase 2 computes d1 = <p_hat, f_hat> (both pre-normalized) and
    d2 = <u, select(mask, f, e)>, then the standard exp/log loss tail;
    per-core partial sums are reduced on host.
"""

import os
import sys

import numpy as np

for _p in ("/opt/trn_rl_repo", os.path.expanduser("~/.axon_site/_ro/trn_rl_repo")):
    if os.path.isdir(_p) and _p not in sys.path:
        sys.path.insert(0, _p)

import ml_dtypes

import concourse.bacc as bacc
import concourse.mybir as mybir
import concourse.tile as tile
from concourse import bass_utils
from concourse.library_config import mlp as mlp_lib


F32 = mybir.dt.float32
BF16 = mybir.dt.bfloat16
I32 = mybir.dt.int32
I16 = mybir.dt.int16
AF = mybir.ActivationFunctionType
ALU = mybir.AluOpType
AX = mybir.AxisListType

NUM_USER = 200000
NUM_ITEM = 200000
DIM_E = 64
DIM_FEAT = 128
B = 16384
G = 17  # 1 + num_neg
TEMP = 0.2
LR_LAMBDA = 0.5

NCORE = 8
BC = B // NCORE          # 2048 batch rows per core
NT = BC // 128           # 16 batch tiles (128 batch rows each)
EPT = 128 * G            # 2176 entries per batch tile
ICOL = EPT // 16         # 136 idx cols per batch tile

# 7 gather banks; int16 gather indices reach < 32768 rows past each base.
BANK = 32768
NBANK = 7

_CACHE: dict = {}

_NEEDED_AF = None


def _patch_act_tables():
    """Force every activation we emit to resolve to the combined
    natural_log_exp_and_others set so no per-anchor table swapping occurs."""
    global _NEEDED_AF
    if _CACHE.get("act_patched"):
        return
    _NEEDED_AF = {AF.Ln, AF.Exp, AF.Prelu, AF.Copy, AF.Identity}
    import concourse.hw_specs as hw_specs
    orig = hw_specs.get_activation_tables

    def patched(module_arch):
        tabs = orig(module_arch)
        out = {}
        for name, fns in tabs.items():
            if name == "natural_log_exp_and_others":
                out[name] = fns
            else:
                out[name] = fns - _NEEDED_AF
        return out

    bacc.get_activation_tables = patched
    _CACHE["act_patched"] = True


def _build(PI, BI, PU, BU):
    """PI / PU: per-bank padded slot counts (multiples of 128); BI / BU: bank
    base row ids. Shared by all cores; sum(PI), sum(PU) <= 32768."""
    _patch_act_tables()
    NBI = sum(PI) // 128          # item slot blocks
    NBI4 = -(-NBI // 4) * 4       # padded to whole encoder chunks
    NBU = sum(PU) // 128          # user slot blocks
    NCH = NBI4 // 4
    assert NBI4 <= 256 and NBU <= 256

    STAGE = int(os.environ.get("KERNEL_STAGE", "3"))
    nc = bacc.Bacc("TRN2", target_bir_lowering=False, debug=False,
                   num_devices=NCORE, num_swdge_queues=4)

    evb_d = nc.dram_tensor("evb", [NUM_ITEM, 256], BF16, kind="ExternalInput")
    idu_d = nc.dram_tensor("idu", [NUM_USER, DIM_E], F32, kind="ExternalInput")
    w1_d = nc.dram_tensor("w1", [DIM_FEAT, 256], BF16, kind="ExternalInput")
    b1_d = nc.dram_tensor("b1", [256], F32, kind="ExternalInput")
    w2_d = nc.dram_tensor("w2", [128, 128], BF16, kind="ExternalInput")
    b2_d = nc.dram_tensor("b2", [DIM_E], F32, kind="ExternalInput")
    i1i_d = nc.dram_tensor("i1i", [128, sum(PI) // 16], I16, kind="ExternalInput")
    i1u_d = nc.dram_tensor("i1u", [128, sum(PU) // 16], I16, kind="ExternalInput")
    i2i_d = nc.dram_tensor("i2i", [128, NT * ICOL], I16, kind="ExternalInput")
    i2u_d = nc.dram_tensor("i2u", [128, NT * ICOL], I16, kind="ExternalInput")
    mk_d = nc.dram_tensor("mask", [NT, 128, G], F32, kind="ExternalInput")
    id_d = nc.dram_tensor("ident", [128, 128], F32, kind="ExternalInput")
    out_d = nc.dram_tensor("acc_out", [128, 2], F32, kind="ExternalOutput")

    scri_d = nc.dram_tensor("scri", [128, NBI4, 128], BF16, kind="Internal")
    scru_d = nc.dram_tensor("scru", [128, NBU, DIM_E], F32, kind="Internal")

    with tile.TileContext(nc) as tc:
        nc.gpsimd.load_library(mlp_lib)
        with tc.tile_pool(name="const", bufs=1) as cp:
            ident = cp.tile([128, 128], F32, tag="ident")
            nc.sync.dma_start(out=ident[:], in_=id_d[:])

            w1sb = cp.tile([128, 256], BF16, tag="w1sb")
            nc.sync.dma_start(out=w1sb[:], in_=w1_d[:])
            w2sb = cp.tile([128, 128], BF16, tag="w2sb")
            nc.sync.dma_start(out=w2sb[:], in_=w2_d[:])
            b1sb = cp.tile([128, 2], F32, tag="b1sb")
            nc.sync.dma_start(out=b1sb[:], in_=b1_d[:].rearrange("(h p) -> p h", p=128))
            b2sb = cp.tile([64, 1], F32, tag="b2sb")
            nc.sync.dma_start(out=b2sb[:], in_=b2_d[:].rearrange("(e o) -> e o", o=1))
            acc = cp.tile([128, 2], F32, tag="acc")
            nc.vector.memset(acc[:], 0.0)

            # single_packet gathers cap at 1024 idxs per call; round-robin the
            # 4 SWDGE queues (each runs on its own Q7 cpu pair -> parallel
            # descriptor generation).
            qrr = [0]

            def gather_split(dst3, blk0, src, idxs, coff0, total, elem):
                done = 0
                while done < total:
                    n = min(1024, total - done)
                    nc.gpsimd.dma_gather(
                        dst3[:, blk0 + done // 128:blk0 + (done + n) // 128, :],
                        src,
                        idxs[:, coff0 + done // 16:coff0 + (done + n) // 16],
                        n, n, elem, queue_num=qrr[0] % 4)
                    qrr[0] += 1
                    done += n

            # ---------------- phase I: items (gather uniques + encode) ------
            with tc.tile_pool(name="l1i", bufs=1) as ip, \
                 tc.tile_pool(name="enc", bufs=3) as ep, \
                 tc.tile_pool(name="l1u", bufs=2) as up, \
                 tc.tile_pool(name="psT", bufs=2, space="PSUM") as psT, \
                 tc.tile_pool(name="psO", bufs=2, space="PSUM") as psO, \
                 tc.tile_pool(name="psH", bufs=2, space="PSUM") as psH, \
                 tc.tile_pool(name="psF", bufs=2, space="PSUM") as psF:

                i1i = ip.tile([128, sum(PI) // 16], I16, tag="i1i")
                nc.sync.dma_start(out=i1i[:], in_=i1i_d[:])
                vdst = ip.tile([128, NBI4 * 256], BF16, tag="vdst")
                vdst3 = vdst[:].rearrange("p (b e) -> p b e", e=256)
                if NBI4 > NBI:
                    nc.vector.memset(vdst3[:, NBI:NBI4, :], 0.0)
                off = 0
                coff = 0
                for b in range(NBANK):
                    gather_split(
                        vdst3, off,
                        evb_d[BI[b]:min(BI[b] + BANK, NUM_ITEM), :],
                        i1i, coff, PI[b], 256)
                    off += PI[b] // 128
                    coff += PI[b] // 16

                # batched per-slot 1/||v||: squares+reduce in 8-block passes,
                # Ln/Exp over 32-block groups (amortizes ACT fixed cost)
                ssall = ip.tile([128, NBI4], F32, tag="ssall")
                j0 = 0
                while j0 < NBI4:
                    nj = min(8, NBI4 - j0)
                    sqt = ep.tile([128, 8 * 128], F32, tag="sqt")
                    sqt3 = sqt[:].rearrange("p (j e) -> p j e", e=DIM_FEAT)
                    nc.vector.tensor_tensor(
                        out=sqt3[:, 0:nj, :], in0=vdst3[:, j0:j0 + nj, 64:192],
                        in1=vdst3[:, j0:j0 + nj, 64:192], op=ALU.mult)
                    nc.vector.tensor_reduce(
                        out=ssall[:, j0:j0 + nj], in_=sqt3[:, 0:nj, :],
                        op=ALU.add, axis=AX.X)
                    j0 += nj
                g0 = 0
                while g0 < NBI4:
                    g1 = min(g0 + 32, NBI4)
                    nc.scalar.activation(out=ssall[:, g0:g1], in_=ssall[:, g0:g1],
                                         func=AF.Ln)
                    nc.scalar.activation(out=ssall[:, g0:g1], in_=ssall[:, g0:g1],
                                         func=AF.Exp, scale=-0.5)
                    g0 = g1

                # user L1 runs concurrently with the encoder
                i1u = up.tile([128, sum(PU) // 16], I16, tag="i1u", bufs=1)
                nc.sync.dma_start(out=i1u[:], in_=i1u_d[:])
                # split banks into ~4 groups to keep the staging tile small
                groups = []
                cur = []
                cum = 0
                for b in range(NBANK):
                    cur.append(b)
                    cum += PU[b] // 128
                    if cum >= NBU // 4:
                        groups.append(cur)
                        cur = []
                        cum = 0
                if cur:
                    groups.append(cur)
                umax = max(sum(PU[b] // 128 for b in g) for g in groups)
                uoff = 0
                ucoff = 0
                for banks in groups:
                    blocks = sum(PU[b] // 128 for b in banks)
                    udst = up.tile([128, umax * DIM_E], F32, tag="udst", bufs=1)
                    ud3 = udst[:].rearrange("p (b e) -> p b e", e=DIM_E)
                    hoff = 0
                    for b in banks:
                        gather_split(
                            ud3, hoff,
                            idu_d[BU[b]:min(BU[b] + BANK, NUM_USER), :],
                            i1u, ucoff, PU[b], DIM_E)
                        hoff += PU[b] // 128
                        ucoff += PU[b] // 16
                    nc.sync.dma_start(
                        out=scru_d[:, uoff:uoff + blocks, :],
                        in_=ud3[:, 0:blocks, :])
                    uoff += blocks

                def enc_chunk(q):
                    v3 = vdst3[:, 4 * q:4 * q + 4, 64:192]   # [128,4,128] bf16
                    vn = ep.tile([128, 512], F32, tag="vn")
                    vn3 = vn[:].rearrange("p (j e) -> p j e", e=DIM_FEAT)
                    nc.vector.tensor_tensor(
                        out=vn3, in0=v3,
                        in1=ssall[:, 4 * q:4 * q + 4]
                            .rearrange("p (j o) -> p j o", o=1)
                            .to_broadcast([128, 4, DIM_FEAT]),
                        op=ALU.mult)
                    vT_ps = psT.tile([128, 512], F32, tag="tp")
                    for j in range(4):
                        nc.tensor.transpose(
                            out=vT_ps[:, j * 128:(j + 1) * 128],
                            in_=vn[:, j * 128:(j + 1) * 128],
                            identity=ident[:])
                    vT = ep.tile([128, 512], BF16, tag="vT")
                    nc.scalar.copy(out=vT[:], in_=vT_ps[:])
                    h_sb = ep.tile([128, 1024], BF16, tag="hsb")
                    for h in range(2):
                        h_ps = psH.tile([128, 512], F32, tag="hp")
                        nc.tensor.matmul(
                            out=h_ps[:], lhsT=w1sb[:, h * 128:(h + 1) * 128],
                            rhs=vT[:], start=True, stop=True)
                        nc.scalar.activation(
                            out=h_sb[:, h * 512:(h + 1) * 512], in_=h_ps[:],
                            func=AF.Prelu, bias=b1sb[:, h:h + 1], scale=1.0,
                            alpha=0.01)
                    f_ps = psF.tile([64, 512], F32, tag="fp")
                    nc.tensor.matmul(out=f_ps[:], lhsT=w2sb[:, 0:64],
                                     rhs=h_sb[:, 0:512], start=True, stop=False)
                    nc.tensor.matmul(out=f_ps[:], lhsT=w2sb[:, 64:128],
                                     rhs=h_sb[:, 512:1024], start=False, stop=True)
                    f_sb = ep.tile([64, 512], F32, tag="fsb")
                    nc.scalar.activation(out=f_sb[:], in_=f_ps[:], func=AF.Identity,
                                         bias=b2sb[:, 0:1])
                    Fo_ps = psO.tile([128, 256], F32, tag="fo")
                    for j in range(4):
                        nc.tensor.transpose(
                            out=Fo_ps[:, j * 64:(j + 1) * 64],
                            in_=f_sb[:, j * 128:(j + 1) * 128],
                            identity=ident[:64, :64])
                    Fo3 = Fo_ps[:].rearrange("p (j e) -> p j e", e=64)
                    sc = ep.tile([128, 4 * 128], BF16, tag="sc", bufs=3)
                    sc3 = sc[:].rearrange("p (j e) -> p j e", e=128)
                    nc.vector.tensor_copy(out=sc3[:, :, 0:64], in_=Fo3)
                    nc.scalar.activation(out=sc3[:, :, 64:128],
                                         in_=vdst3[:, 4 * q:4 * q + 4, 0:64],
                                         func=AF.Copy)
                    nc.sync.dma_start(out=scri_d[:, 4 * q:4 * q + 4, :], in_=sc3)

                for q in range(NCH if STAGE >= 1 else 0):
                    enc_chunk(q)

            # ---------------- phase III: per-entry streams + loss ----------
            with tc.tile_pool(name="ph2", bufs=3) as pp, \
                 tc.tile_pool(name="prd", bufs=3) as prp:
                i2i = pp.tile([128, NT * ICOL], I16, tag="i2i", bufs=1)
                nc.sync.dma_start(out=i2i[:], in_=i2i_d[:])
                i2u = pp.tile([128, NT * ICOL], I16, tag="i2u", bufs=1)
                nc.sync.dma_start(out=i2u[:], in_=i2u_d[:])
                rows_i = scri_d[:].rearrange("p b e -> (p b) e")
                rows_u = scru_d[:].rearrange("p b e -> (p b) e")

                def ph2_pair(bp):
                    """Gather two batch-tiles' entries per split-gather
                    (4352 idxs = 4x1024 + 256 -> 5 calls/table)."""
                    ti_p = pp.tile([128, 2 * G * 128], BF16, tag="ti", bufs=2)
                    ti_p3 = ti_p[:].rearrange("p (g e) -> p g e", e=128)
                    gather_split(ti_p3, 0, rows_i, i2i, 2 * bp * ICOL,
                                 2 * EPT, 128)
                    tu_p = pp.tile([128, 2 * G * 64], F32, tag="tu", bufs=2)
                    tu_p3 = tu_p[:].rearrange("p (g e) -> p g e", e=64)
                    gather_split(tu_p3, 0, rows_u, i2u, 2 * bp * ICOL,
                                 2 * EPT, DIM_E)
                    for half in range(2):
                        ph2_tile(2 * bp + half,
                                 ti_p[:, half * G * 128:(half + 1) * G * 128],
                                 tu_p[:, half * G * 64:(half + 1) * G * 64])

                def ph2_tile(bt, ti, tu):
                    ti3 = ti.rearrange("p (g e) -> p g e", e=128)
                    tu3 = tu.rearrange("p (g e) -> p g e", e=64)
                    mk = pp.tile([128, G], F32, tag="mk")
                    nc.sync.dma_start(out=mk[:], in_=mk_d[bt])

                    F3 = ti3[:, :, 0:64]
                    E3 = ti3[:, :, 64:128]

                    # p_hat from the positive item's raw embedding (g=0 slot)
                    E0 = ti[:, 64:128]
                    prP = pp.tile([128, 64], F32, tag="prP")
                    dPP = pp.tile([128, 1], F32, tag="dPP")
                    nc.vector.tensor_tensor(out=prP[:], in0=E0, in1=E0, op=ALU.mult)
                    nc.vector.tensor_reduce(
                        out=dPP[:], in_=prP[:].rearrange("p (o e) -> p o e", o=1),
                        op=ALU.add, axis=AX.X)
                    nc.scalar.activation(out=dPP[:], in_=dPP[:], func=AF.Ln)
                    nc.scalar.activation(out=dPP[:], in_=dPP[:], func=AF.Exp,
                                         scale=-0.5)
                    ph = pp.tile([128, 64], BF16, tag="ph")
                    nc.vector.tensor_tensor(
                        out=ph[:].rearrange("p (o e) -> p o e", o=1),
                        in0=E0.rearrange("p (o e) -> p o e", o=1),
                        in1=dPP[:].rearrange("p (o e) -> p o e", e=1)
                            .to_broadcast([128, 1, 64]),
                        op=ALU.mult)
                    ph3b = ph[:].rearrange("p (g e) -> p g e", g=1) \
                        .to_broadcast([128, G, 64])

                    # d1 = <p_hat, f> / ||f||  (f left unnormalized in scratch)
                    prA = prp.tile([128, G * 64], F32, tag="prA")
                    prA3 = prA[:].rearrange("p (g e) -> p g e", e=64)
                    dPF = pp.tile([128, G], F32, tag="dPF")
                    nc.vector.tensor_tensor(out=prA3, in0=F3, in1=ph3b, op=ALU.mult)
                    nc.vector.tensor_reduce(out=dPF[:], in_=prA3, op=ALU.add,
                                            axis=AX.X)
                    prB = prp.tile([128, G * 64], F32, tag="prB")
                    prB3 = prB[:].rearrange("p (g e) -> p g e", e=64)
                    nf2 = pp.tile([128, G], F32, tag="nf2")
                    nc.vector.tensor_tensor(out=prB3, in0=F3, in1=F3, op=ALU.mult)
                    nc.vector.tensor_reduce(out=nf2[:], in_=prB3, op=ALU.add,
                                            axis=AX.X)
                    nc.scalar.activation(out=nf2[:], in_=nf2[:], func=AF.Ln)
                    nc.scalar.activation(out=nf2[:], in_=nf2[:], func=AF.Exp,
                                         scale=-0.5)
                    d1 = pp.tile([128, G], F32, tag="d1")
                    nc.vector.tensor_tensor(out=d1[:], in0=dPF[:], in1=nf2[:],
                                            op=ALU.mult)

                    # d2 = <u, e> + m * (<u, f> - <u, e>)  (scalar select)
                    prC = prp.tile([128, G * 64], F32, tag="prC")
                    prC3 = prC[:].rearrange("p (g e) -> p g e", e=64)
                    dA = pp.tile([128, G], F32, tag="dA")
                    nc.vector.tensor_tensor(out=prC3, in0=tu3, in1=E3, op=ALU.mult)
                    nc.vector.tensor_reduce(out=dA[:], in_=prC3, op=ALU.add, axis=AX.X)
                    prD = prp.tile([128, G * 64], F32, tag="prD")
                    prD3 = prD[:].rearrange("p (g e) -> p g e", e=64)
                    dB = pp.tile([128, G], F32, tag="dB")
                    nc.vector.tensor_tensor(out=prD3, in0=tu3, in1=F3, op=ALU.mult)
                    nc.vector.tensor_reduce(out=dB[:], in_=prD3, op=ALU.add, axis=AX.X)
                    d2 = pp.tile([128, G], F32, tag="d2")
                    nc.vector.tensor_tensor(out=d2[:], in0=dB[:], in1=dA[:],
                                            op=ALU.subtract)
                    nc.vector.tensor_tensor(out=d2[:], in0=d2[:], in1=mk[:],
                                            op=ALU.mult)
                    nc.vector.tensor_tensor(out=d2[:], in0=d2[:], in1=dA[:],
                                            op=ALU.add)

                    # loss tail: -log(pos/tot) = ln(tot) - d[:,0]/T
                    s1 = pp.tile([128, G], F32, tag="s1")
                    s2 = pp.tile([128, G], F32, tag="s2")
                    tot = pp.tile([128, 2], F32, tag="tot")
                    nc.scalar.activation(out=s1[:], in_=d1[:], func=AF.Exp,
                                         scale=1.0 / TEMP, accum_out=tot[:, 0:1])
                    nc.scalar.activation(out=s2[:], in_=d2[:], func=AF.Exp,
                                         scale=1.0 / TEMP, accum_out=tot[:, 1:2])
                    nc.scalar.activation(out=tot[:], in_=tot[:], func=AF.Ln)
                    dd = pp.tile([128, 2], F32, tag="dd")
                    nc.scalar.activation(out=dd[:, 0:1], in_=d1[:, 0:1],
                                         func=AF.Copy, scale=-1.0 / TEMP)
                    nc.scalar.activation(out=dd[:, 1:2], in_=d2[:, 0:1],
                                         func=AF.Copy, scale=-1.0 / TEMP)
                    nc.vector.tensor_tensor(out=acc[:], in0=acc[:], in1=tot[:],
                                            op=ALU.add)
                    nc.vector.tensor_tensor(out=acc[:], in0=acc[:], in1=dd[:],
                                            op=ALU.add)

                for bp in range(NT // 2 if STAGE >= 3 else 0):
                    ph2_pair(bp)

            nc.sync.dma_start(out=out_d[:], in_=acc[:])

    nc.compile()
    return nc


def _wrap_idx(idx):
    """idx list -> [128, ceil(n/16)] int16 wrapped (idx i at [i%16, i//16]),
    replicated across the 8 groups of 16 partitions."""
    idx = np.asarray(idx, np.int16)
    n = len(idx)
    cols = -(-n // 16)
    if cols * 16 > n:
        fill = idx[-1] if n else np.int16(0)
        idx = np.concatenate([idx, np.full(cols * 16 - n, fill, np.int16)])
    w = np.ascontiguousarray(idx.reshape(cols, 16).T)
    return np.tile(w, (8, 1))


def _bank_plan(uniqs):
    """Position-based banking: slot space = ceil(max_core_uniques/128)*128
    slots split into 7 contiguous bank segments; bank b of core c takes the
    core's sorted unique ids at positions [C_b, C_b + n) (n <= P_b). Bank base
    = min over cores of the id at position C_b, so every relative index is in
    [0, 32768)."""
    smax = max(len(u) for u in uniqs)
    nbt = -(-smax // 128)
    k, r = divmod(nbt, NBANK)
    blocks = [k + 1] * r + [k] * (NBANK - r)
    P = [bl * 128 for bl in blocks]
    C = np.concatenate([[0], np.cumsum(P)])
    bases = []
    for b in range(NBANK):
        lo = min(int(u[C[b]]) for u in uniqs if C[b] < len(u))
        bases.append(lo)
        for u in uniqs:
            end = min(C[b + 1], len(u))
            if end > C[b]:
                assert int(u[end - 1]) - lo < BANK, "bank window overflow"
    return P, bases


def _host_prep(user_tensor, item_tensor, rand_index):
    it = np.clip(item_tensor.astype(np.int64) - NUM_USER, 0, NUM_ITEM - 1)
    ut = user_tensor.astype(np.int64)
    mask = np.zeros(B * G, np.float32)
    mask[np.asarray(rand_index, dtype=np.int64)] = 1.0
    mask = mask.reshape(B, G)

    cores = []
    for k in range(NCORE):
        sl = slice(k * BC, (k + 1) * BC)
        itc, utc = it[sl], ut[sl]
        cores.append((itc, utc, mask[sl], np.unique(itc), np.unique(utc)))

    PI, BI = _bank_plan([c[3] for c in cores])
    PU, BU = _bank_plan([c[4] for c in cores])
    assert sum(PI) <= BANK and sum(PU) <= BANK, (PI, PU)
    NBI4 = -(-(sum(PI) // 128) // 4) * 4
    NBU = sum(PU) // 128

    def idx_arrays(u, P, bases, nb_total):
        """Returns (l1_idx_wrapped, j_of_unique) for one core."""
        C = np.concatenate([[0], np.cumsum(P)])
        l1 = np.zeros(sum(P), np.int16)
        slots = np.zeros(len(u), np.int64)
        for b in range(NBANK):
            pos, end = int(C[b]), min(int(C[b + 1]), len(u))
            n = max(end - pos, 0)
            ids = u[pos:end]
            rel = (ids - bases[b]).astype(np.int64)
            assert n == 0 or (rel.min() >= 0 and rel.max() < BANK)
            l1[pos:pos + n] = rel.astype(np.int16)
            if n < P[b]:
                l1[pos + n:C[b + 1]] = l1[pos + n - 1] if n else 0
            slots[pos:end] = pos + np.arange(n)
        j = (slots % 128) * nb_total + slots // 128
        assert len(j) == 0 or j.max() < 32768
        return _wrap_idx(l1), j.astype(np.int16)

    per_core = []
    for (itc, utc, mkc, ui, uu) in cores:
        l1i, ji = idx_arrays(ui, PI, BI, NBI4)
        l1u, ju = idx_arrays(uu, PU, BU, NBU)
        # L2 idx in g-major order per batch tile: i = g*128 + r
        je = ji[np.searchsorted(ui, itc)]       # [2048, 17]
        jue = ju[np.searchsorted(uu, utc)]
        l2i = je.reshape(NT, 128, G).transpose(0, 2, 1).reshape(NT, EPT)
        l2u = jue.reshape(NT, 128, G).transpose(0, 2, 1).reshape(NT, EPT)

        def wrap_l2(l2):
            out = np.zeros((16, NT * ICOL), np.int16)
            for bt in range(NT):
                out[:, bt * ICOL:(bt + 1) * ICOL] = \
                    l2[bt].reshape(ICOL, 16).T
            return np.tile(out, (8, 1))

        per_core.append({
            "i1i": l1i, "i1u": l1u,
            "i2i": wrap_l2(l2i), "i2u": wrap_l2(l2u),
            "mask": np.ascontiguousarray(mkc.reshape(NT, 128, G)),
        })
    return PI, BI, PU, BU, per_core


def kernel(v_feat, id_embedding, W1, b1, W2, b2, user_tensor, item_tensor,
           rand_index):
    v_feat = np.asarray(v_feat, dtype=np.float32)
    id_embedding = np.asarray(id_embedding, dtype=np.float32)
    # evb row: [e(0:64) | v(64:192) | 0(192:256)] bf16
    evb = np.zeros((NUM_ITEM, 256), ml_dtypes.bfloat16)
    evb[:, 0:64] = id_embedding[NUM_USER:]
    evb[:, 64:192] = v_feat
    idu = np.ascontiguousarray(id_embedding[:NUM_USER])
    W1b = np.ascontiguousarray(W1, dtype=np.float32).astype(ml_dtypes.bfloat16)
    b1f = np.ascontiguousarray(b1, dtype=np.float32)
    W2f = np.ascontiguousarray(W2, dtype=np.float32)
    # pack W2 halves side by side: [128, 0:64] = W2[0:128], [128, 64:128] = W2[128:256]
    W2b = np.concatenate([W2f[0:128, :], W2f[128:256, :]], axis=1) \
        .astype(ml_dtypes.bfloat16)
    b2f = np.ascontiguousarray(b2, dtype=np.float32)

    PI, BI, PU, BU, per_core = _host_prep(user_tensor, item_tensor, rand_index)

    key = (tuple(PI), tuple(BI), tuple(PU), tuple(BU))
    if _CACHE.get("key") != key:
        _CACHE["nc"] = _build(PI, BI, PU, BU)
        _CACHE["key"] = key
    nc = _CACHE["nc"]

    in_maps = []
    for k in range(NCORE):
        m = {"evb": evb, "idu": idu, "w1": W1b, "b1": b1f, "w2": W2b,
             "b2": b2f, "ident": np.eye(128, dtype=np.float32)}
        m.update(per_core[k])
        in_maps.append(m)
    trace = bool(int(os.environ.get("KERNEL_TRACE", "0")))
    res = bass_utils.run_bass_kernel_spmd(
        nc, in_maps, core_ids=list(range(NCORE)), trace=trace)
    _CACHE["last_results"] = res
    accs = np.stack([r["acc_out"] for r in res.results])  # [8, 128, 2]
    sums = accs.sum(axis=(0, 1), dtype=np.float64)
    l1 = sums[0] / B
    l2 = sums[1] / B
    return np.array(LR_LAMBDA * l1 + (1.0 - LR_LAMBDA) * l2, dtype=np.float32)


# revision 60
# speedup vs baseline: 1.3669x; 1.0770x over previous
"""Trainium2 Bass kernel for nn_CLCRec contrastive loss (v2: dma_gather).

Strategy (fully local per core, no collectives):
  - Batch rows sharded 8 ways (2048 rows x 17 group entries per core).
  - All table row fetches use SWDGE dma_gather (0.34ns/desc issue) instead of
    per-128-row indirect DMAs (994ns fixed each); this removes the Pool-engine
    SWDGE-issue bottleneck (655us busy in v1).
  - dma_gather indices are int16 (<32768), so gathers are two-level:
      L1: the core's ~32k *unique* item/user ids, sorted (so ids are
          bank-contiguous), gathered bank-by-bank (7 banks span 200000 rows)
          into a dense relabeled slot space.
      Items are encoded (bf16 MLP), assembled into scratch rows
          [f_hat | f | e | pad] (512B) and streamed to an HBM scratch table;
          user rows go to a second f32 scratch table.
      L2: per batch-tile, ONE gather per table fetches the per-entry rows
          from scratch (slot count <= 32768 fits int16) in g-major order so
          the dst lands as [128 batch rows, 17, elems].
  - Phase 2 computes d1 = <p_hat, f_hat> (both pre-normalized) and
    d2 = <u, select(mask, f, e)>, then the standard exp/log loss tail;
    per-core partial sums are reduced on host.
"""

import os
import sys

import numpy as np

for _p in ("/opt/trn_rl_repo", os.path.expanduser("~/.axon_site/_ro/trn_rl_repo")):
    if os.path.isdir(_p) and _p not in sys.path:
        sys.path.insert(0, _p)

import ml_dtypes

import concourse.bacc as bacc
import concourse.mybir as mybir
import concourse.tile as tile
from concourse import bass_utils
from concourse.library_config import mlp as mlp_lib


F32 = mybir.dt.float32
BF16 = mybir.dt.bfloat16
I32 = mybir.dt.int32
I16 = mybir.dt.int16
AF = mybir.ActivationFunctionType
ALU = mybir.AluOpType
AX = mybir.AxisListType

NUM_USER = 200000
NUM_ITEM = 200000
DIM_E = 64
DIM_FEAT = 128
B = 16384
G = 17  # 1 + num_neg
TEMP = 0.2
LR_LAMBDA = 0.5

NCORE = 8
BC = B // NCORE          # 2048 batch rows per core
NT = BC // 128           # 16 batch tiles (128 batch rows each)
EPT = 128 * G            # 2176 entries per batch tile
ICOL = EPT // 16         # 136 idx cols per batch tile

# 7 gather banks; int16 gather indices reach < 32768 rows past each base.
BANK = 32768
NBANK = 7

_CACHE: dict = {}

_NEEDED_AF = None


def _patch_act_tables():
    """Force every activation we emit to resolve to the combined
    natural_log_exp_and_others set so no per-anchor table swapping occurs."""
    global _NEEDED_AF
    if _CACHE.get("act_patched"):
        return
    _NEEDED_AF = {AF.Ln, AF.Exp, AF.Prelu, AF.Copy, AF.Identity}
    import concourse.hw_specs as hw_specs
    orig = hw_specs.get_activation_tables

    def patched(module_arch):
        tabs = orig(module_arch)
        out = {}
        for name, fns in tabs.items():
            if name == "natural_log_exp_and_others":
                out[name] = fns
            else:
                out[name] = fns - _NEEDED_AF
        return out

    bacc.get_activation_tables = patched
    _CACHE["act_patched"] = True


def _build(PI, BI, PU, BU):
    """PI / PU: per-bank padded slot counts (multiples of 128); BI / BU: bank
    base row ids. Shared by all cores; sum(PI), sum(PU) <= 32768."""
    _patch_act_tables()
    NBI = sum(PI) // 128          # item slot blocks
    NBI4 = -(-NBI // 4) * 4       # padded to whole encoder chunks
    NBU = sum(PU) // 128          # user slot blocks
    NCH = NBI4 // 4
    assert NBI4 <= 256 and NBU <= 256

    STAGE = int(os.environ.get("KERNEL_STAGE", "3"))
    nc = bacc.Bacc("TRN2", target_bir_lowering=False, debug=False,
                   num_devices=NCORE, num_swdge_queues=4)

    evb_d = nc.dram_tensor("evb", [NUM_ITEM, 256], BF16, kind="ExternalInput")
    idu_d = nc.dram_tensor("idu", [NUM_USER, DIM_E], F32, kind="ExternalInput")
    w1_d = nc.dram_tensor("w1", [DIM_FEAT, 256], BF16, kind="ExternalInput")
    b1_d = nc.dram_tensor("b1", [256], F32, kind="ExternalInput")
    w2_d = nc.dram_tensor("w2", [128, 128], BF16, kind="ExternalInput")
    b2_d = nc.dram_tensor("b2", [DIM_E], F32, kind="ExternalInput")
    i1i_d = nc.dram_tensor("i1i", [128, sum(PI) // 16], I16, kind="ExternalInput")
    i1u_d = nc.dram_tensor("i1u", [128, sum(PU) // 16], I16, kind="ExternalInput")
    i2i_d = nc.dram_tensor("i2i", [128, NT * ICOL], I16, kind="ExternalInput")
    i2u_d = nc.dram_tensor("i2u", [128, NT * ICOL], I16, kind="ExternalInput")
    mk_d = nc.dram_tensor("mask", [NT, 128, G], F32, kind="ExternalInput")
    id_d = nc.dram_tensor("ident", [128, 128], F32, kind="ExternalInput")
    out_d = nc.dram_tensor("acc_out", [128, 2], F32, kind="ExternalOutput")

    scri_d = nc.dram_tensor("scri", [128, NBI4, 128], BF16, kind="Internal")
    scru_d = nc.dram_tensor("scru", [128, NBU, DIM_E], F32, kind="Internal")

    with tile.TileContext(nc) as tc:
        nc.gpsimd.load_library(mlp_lib)
        with tc.tile_pool(name="const", bufs=1) as cp:
            ident = cp.tile([128, 128], F32, tag="ident")
            nc.sync.dma_start(out=ident[:], in_=id_d[:])

            w1sb = cp.tile([128, 256], BF16, tag="w1sb")
            nc.sync.dma_start(out=w1sb[:], in_=w1_d[:])
            w2sb = cp.tile([128, 128], BF16, tag="w2sb")
            nc.sync.dma_start(out=w2sb[:], in_=w2_d[:])
            b1sb = cp.tile([128, 2], F32, tag="b1sb")
            nc.sync.dma_start(out=b1sb[:], in_=b1_d[:].rearrange("(h p) -> p h", p=128))
            b2sb = cp.tile([64, 1], F32, tag="b2sb")
            nc.sync.dma_start(out=b2sb[:], in_=b2_d[:].rearrange("(e o) -> e o", o=1))
            acc = cp.tile([128, 2], F32, tag="acc")
            nc.vector.memset(acc[:], 0.0)

            # single_packet gathers cap at 1024 idxs per call; round-robin the
            # 4 SWDGE queues (each runs on its own Q7 cpu pair -> parallel
            # descriptor generation).
            qrr = [0]

            def gather_split(dst3, blk0, src, idxs, coff0, total, elem):
                done = 0
                while done < total:
                    n = min(1024, total - done)
                    nc.gpsimd.dma_gather(
                        dst3[:, blk0 + done // 128:blk0 + (done + n) // 128, :],
                        src,
                        idxs[:, coff0 + done // 16:coff0 + (done + n) // 16],
                        n, n, elem, queue_num=qrr[0] % 4)
                    qrr[0] += 1
                    done += n

            # ---------------- phase I: items (gather uniques + encode) ------
            with tc.tile_pool(name="l1i", bufs=1) as ip, \
                 tc.tile_pool(name="enc", bufs=3) as ep, \
                 tc.tile_pool(name="l1u", bufs=2) as up, \
                 tc.tile_pool(name="psT", bufs=2, space="PSUM") as psT, \
                 tc.tile_pool(name="psO", bufs=2, space="PSUM") as psO, \
                 tc.tile_pool(name="psH", bufs=2, space="PSUM") as psH, \
                 tc.tile_pool(name="psF", bufs=2, space="PSUM") as psF:

                i1i = ip.tile([128, sum(PI) // 16], I16, tag="i1i")
                nc.sync.dma_start(out=i1i[:], in_=i1i_d[:])
                vdst = ip.tile([128, NBI4 * 256], BF16, tag="vdst")
                vdst3 = vdst[:].rearrange("p (b e) -> p b e", e=256)
                if NBI4 > NBI:
                    nc.vector.memset(vdst3[:, NBI:NBI4, :], 0.0)
                off = 0
                coff = 0
                for b in range(NBANK):
                    gather_split(
                        vdst3, off,
                        evb_d[BI[b]:min(BI[b] + BANK, NUM_ITEM), :],
                        i1i, coff, PI[b], 256)
                    off += PI[b] // 128
                    coff += PI[b] // 16

                # batched per-slot 1/||v||: squares+reduce in 8-block passes,
                # Ln/Exp over 32-block groups, emitted interleaved with the
                # encoder chunks so chunk 0 doesn't wait on late bank gathers
                ssall = ip.tile([128, NBI4], F32, tag="ssall")

                def ssall_group(blk0):
                    end = min(blk0 + 32, NBI4)
                    j0 = blk0
                    while j0 < end:
                        nj = min(8, end - j0)
                        sqt = ep.tile([128, 8 * 128], F32, tag="sqt")
                        sqt3 = sqt[:].rearrange("p (j e) -> p j e", e=DIM_FEAT)
                        nc.vector.tensor_tensor(
                            out=sqt3[:, 0:nj, :],
                            in0=vdst3[:, j0:j0 + nj, 64:192],
                            in1=vdst3[:, j0:j0 + nj, 64:192], op=ALU.mult)
                        nc.vector.tensor_reduce(
                            out=ssall[:, j0:j0 + nj], in_=sqt3[:, 0:nj, :],
                            op=ALU.add, axis=AX.X)
                        j0 += nj
                    nc.scalar.activation(out=ssall[:, blk0:end],
                                         in_=ssall[:, blk0:end], func=AF.Ln)
                    nc.scalar.activation(out=ssall[:, blk0:end],
                                         in_=ssall[:, blk0:end],
                                         func=AF.Exp, scale=-0.5)

                # user L1 runs concurrently with the encoder
                i1u = up.tile([128, sum(PU) // 16], I16, tag="i1u", bufs=1)
                nc.sync.dma_start(out=i1u[:], in_=i1u_d[:])
                # split banks into ~4 groups to keep the staging tile small
                groups = []
                cur = []
                cum = 0
                for b in range(NBANK):
                    cur.append(b)
                    cum += PU[b] // 128
                    if cum >= NBU // 4:
                        groups.append(cur)
                        cur = []
                        cum = 0
                if cur:
                    groups.append(cur)
                umax = max(sum(PU[b] // 128 for b in g) for g in groups)
                uoff = 0
                ucoff = 0
                for banks in groups:
                    blocks = sum(PU[b] // 128 for b in banks)
                    udst = up.tile([128, umax * DIM_E], F32, tag="udst", bufs=1)
                    ud3 = udst[:].rearrange("p (b e) -> p b e", e=DIM_E)
                    hoff = 0
                    for b in banks:
                        gather_split(
                            ud3, hoff,
                            idu_d[BU[b]:min(BU[b] + BANK, NUM_USER), :],
                            i1u, ucoff, PU[b], DIM_E)
                        hoff += PU[b] // 128
                        ucoff += PU[b] // 16
                    nc.sync.dma_start(
                        out=scru_d[:, uoff:uoff + blocks, :],
                        in_=ud3[:, 0:blocks, :])
                    uoff += blocks

                def enc_chunk(q):
                    v3 = vdst3[:, 4 * q:4 * q + 4, 64:192]   # [128,4,128] bf16
                    vn = ep.tile([128, 512], F32, tag="vn")
                    vn3 = vn[:].rearrange("p (j e) -> p j e", e=DIM_FEAT)
                    nc.vector.tensor_tensor(
                        out=vn3, in0=v3,
                        in1=ssall[:, 4 * q:4 * q + 4]
                            .rearrange("p (j o) -> p j o", o=1)
                            .to_broadcast([128, 4, DIM_FEAT]),
                        op=ALU.mult)
                    vT_ps = psT.tile([128, 512], F32, tag="tp")
                    for j in range(4):
                        nc.tensor.transpose(
                            out=vT_ps[:, j * 128:(j + 1) * 128],
                            in_=vn[:, j * 128:(j + 1) * 128],
                            identity=ident[:])
                    vT = ep.tile([128, 512], BF16, tag="vT")
                    nc.scalar.copy(out=vT[:], in_=vT_ps[:])
                    h_sb = ep.tile([128, 1024], BF16, tag="hsb")
                    for h in range(2):
                        h_ps = psH.tile([128, 512], F32, tag="hp")
                        nc.tensor.matmul(
                            out=h_ps[:], lhsT=w1sb[:, h * 128:(h + 1) * 128],
                            rhs=vT[:], start=True, stop=True)
                        nc.scalar.activation(
                            out=h_sb[:, h * 512:(h + 1) * 512], in_=h_ps[:],
                            func=AF.Prelu, bias=b1sb[:, h:h + 1], scale=1.0,
                            alpha=0.01)
                    f_ps = psF.tile([64, 512], F32, tag="fp")
                    nc.tensor.matmul(out=f_ps[:], lhsT=w2sb[:, 0:64],
                                     rhs=h_sb[:, 0:512], start=True, stop=False)
                    nc.tensor.matmul(out=f_ps[:], lhsT=w2sb[:, 64:128],
                                     rhs=h_sb[:, 512:1024], start=False, stop=True)
                    f_sb = ep.tile([64, 512], F32, tag="fsb")
                    nc.scalar.activation(out=f_sb[:], in_=f_ps[:], func=AF.Identity,
                                         bias=b2sb[:, 0:1])
                    Fo_ps = psO.tile([128, 256], F32, tag="fo")
                    for j in range(4):
                        nc.tensor.transpose(
                            out=Fo_ps[:, j * 64:(j + 1) * 64],
                            in_=f_sb[:, j * 128:(j + 1) * 128],
                            identity=ident[:64, :64])
                    Fo3 = Fo_ps[:].rearrange("p (j e) -> p j e", e=64)
                    sc = ep.tile([128, 4 * 128], BF16, tag="sc", bufs=3)
                    sc3 = sc[:].rearrange("p (j e) -> p j e", e=128)
                    nc.vector.tensor_copy(out=sc3[:, :, 0:64], in_=Fo3)
                    nc.scalar.activation(out=sc3[:, :, 64:128],
                                         in_=vdst3[:, 4 * q:4 * q + 4, 0:64],
                                         func=AF.Copy)
                    nc.sync.dma_start(out=scri_d[:, 4 * q:4 * q + 4, :], in_=sc3)

                for q in range(NCH if STAGE >= 1 else 0):
                    if (4 * q) % 32 == 0:
                        ssall_group(4 * q)
                    enc_chunk(q)

            # ---------------- phase III: per-entry streams + loss ----------
            with tc.tile_pool(name="ph2", bufs=3) as pp, \
                 tc.tile_pool(name="prd", bufs=3) as prp:
                i2i = pp.tile([128, NT * ICOL], I16, tag="i2i", bufs=1)
                nc.sync.dma_start(out=i2i[:], in_=i2i_d[:])
                i2u = pp.tile([128, NT * ICOL], I16, tag="i2u", bufs=1)
                nc.sync.dma_start(out=i2u[:], in_=i2u_d[:])
                rows_i = scri_d[:].rearrange("p b e -> (p b) e")
                rows_u = scru_d[:].rearrange("p b e -> (p b) e")

                def ph2_pair(bp):
                    """Gather four batch-tiles' entries per split-gather
                    (8704 idxs = 8x1024 + 512 -> 9 calls/table)."""
                    ti_p = pp.tile([128, 4 * G * 128], BF16, tag="ti", bufs=2)
                    ti_p3 = ti_p[:].rearrange("p (g e) -> p g e", e=128)
                    gather_split(ti_p3, 0, rows_i, i2i, 4 * bp * ICOL,
                                 4 * EPT, 128)
                    tu_p = pp.tile([128, 4 * G * 64], F32, tag="tu", bufs=2)
                    tu_p3 = tu_p[:].rearrange("p (g e) -> p g e", e=64)
                    gather_split(tu_p3, 0, rows_u, i2u, 4 * bp * ICOL,
                                 4 * EPT, DIM_E)
                    for half in range(4):
                        ph2_tile(4 * bp + half,
                                 ti_p[:, half * G * 128:(half + 1) * G * 128],
                                 tu_p[:, half * G * 64:(half + 1) * G * 64])

                def ph2_tile(bt, ti, tu):
                    ti3 = ti.rearrange("p (g e) -> p g e", e=128)
                    tu3 = tu.rearrange("p (g e) -> p g e", e=64)
                    mk = pp.tile([128, G], F32, tag="mk")
                    nc.sync.dma_start(out=mk[:], in_=mk_d[bt])

                    F3 = ti3[:, :, 0:64]
                    E3 = ti3[:, :, 64:128]

                    # p_hat from the positive item's raw embedding (g=0 slot)
                    E0 = ti[:, 64:128]
                    prP = pp.tile([128, 64], F32, tag="prP")
                    dPP = pp.tile([128, 1], F32, tag="dPP")
                    nc.vector.tensor_tensor(out=prP[:], in0=E0, in1=E0, op=ALU.mult)
                    nc.vector.tensor_reduce(
                        out=dPP[:], in_=prP[:].rearrange("p (o e) -> p o e", o=1),
                        op=ALU.add, axis=AX.X)
                    nc.scalar.activation(out=dPP[:], in_=dPP[:], func=AF.Ln)
                    nc.scalar.activation(out=dPP[:], in_=dPP[:], func=AF.Exp,
                                         scale=-0.5)
                    ph = pp.tile([128, 64], BF16, tag="ph")
                    nc.vector.tensor_tensor(
                        out=ph[:].rearrange("p (o e) -> p o e", o=1),
                        in0=E0.rearrange("p (o e) -> p o e", o=1),
                        in1=dPP[:].rearrange("p (o e) -> p o e", e=1)
                            .to_broadcast([128, 1, 64]),
                        op=ALU.mult)
                    ph3b = ph[:].rearrange("p (g e) -> p g e", g=1) \
                        .to_broadcast([128, G, 64])

                    # d1 = <p_hat, f> / ||f||  (f left unnormalized in scratch)
                    prA = prp.tile([128, G * 64], F32, tag="prA")
                    prA3 = prA[:].rearrange("p (g e) -> p g e", e=64)
                    dPF = pp.tile([128, G], F32, tag="dPF")
                    nc.vector.tensor_tensor(out=prA3, in0=F3, in1=ph3b, op=ALU.mult)
                    nc.vector.tensor_reduce(out=dPF[:], in_=prA3, op=ALU.add,
                                            axis=AX.X)
                    prB = prp.tile([128, G * 64], F32, tag="prB")
                    prB3 = prB[:].rearrange("p (g e) -> p g e", e=64)
                    nf2 = pp.tile([128, G], F32, tag="nf2")
                    nc.vector.tensor_tensor(out=prB3, in0=F3, in1=F3, op=ALU.mult)
                    nc.vector.tensor_reduce(out=nf2[:], in_=prB3, op=ALU.add,
                                            axis=AX.X)
                    nc.scalar.activation(out=nf2[:], in_=nf2[:], func=AF.Ln)
                    nc.scalar.activation(out=nf2[:], in_=nf2[:], func=AF.Exp,
                                         scale=-0.5)
                    d1 = pp.tile([128, G], F32, tag="d1")
                    nc.vector.tensor_tensor(out=d1[:], in0=dPF[:], in1=nf2[:],
                                            op=ALU.mult)

                    # d2 = <u, e> + m * (<u, f> - <u, e>)  (scalar select)
                    prC = prp.tile([128, G * 64], F32, tag="prC")
                    prC3 = prC[:].rearrange("p (g e) -> p g e", e=64)
                    dA = pp.tile([128, G], F32, tag="dA")
                    nc.vector.tensor_tensor(out=prC3, in0=tu3, in1=E3, op=ALU.mult)
                    nc.vector.tensor_reduce(out=dA[:], in_=prC3, op=ALU.add, axis=AX.X)
                    prD = prp.tile([128, G * 64], F32, tag="prD")
                    prD3 = prD[:].rearrange("p (g e) -> p g e", e=64)
                    dB = pp.tile([128, G], F32, tag="dB")
                    nc.vector.tensor_tensor(out=prD3, in0=tu3, in1=F3, op=ALU.mult)
                    nc.vector.tensor_reduce(out=dB[:], in_=prD3, op=ALU.add, axis=AX.X)
                    d2 = pp.tile([128, G], F32, tag="d2")
                    nc.vector.tensor_tensor(out=d2[:], in0=dB[:], in1=dA[:],
                                            op=ALU.subtract)
                    nc.vector.tensor_tensor(out=d2[:], in0=d2[:], in1=mk[:],
                                            op=ALU.mult)
                    nc.vector.tensor_tensor(out=d2[:], in0=d2[:], in1=dA[:],
                                            op=ALU.add)

                    # loss tail: -log(pos/tot) = ln(tot) - d[:,0]/T
                    s1 = pp.tile([128, G], F32, tag="s1")
                    s2 = pp.tile([128, G], F32, tag="s2")
                    tot = pp.tile([128, 2], F32, tag="tot")
                    nc.scalar.activation(out=s1[:], in_=d1[:], func=AF.Exp,
                                         scale=1.0 / TEMP, accum_out=tot[:, 0:1])
                    nc.scalar.activation(out=s2[:], in_=d2[:], func=AF.Exp,
                                         scale=1.0 / TEMP, accum_out=tot[:, 1:2])
                    nc.scalar.activation(out=tot[:], in_=tot[:], func=AF.Ln)
                    dd = pp.tile([128, 2], F32, tag="dd")
                    nc.scalar.activation(out=dd[:, 0:1], in_=d1[:, 0:1],
                                         func=AF.Copy, scale=-1.0 / TEMP)
                    nc.scalar.activation(out=dd[:, 1:2], in_=d2[:, 0:1],
                                         func=AF.Copy, scale=-1.0 / TEMP)
                    nc.vector.tensor_tensor(out=acc[:], in0=acc[:], in1=tot[:],
                                            op=ALU.add)
                    nc.vector.tensor_tensor(out=acc[:], in0=acc[:], in1=dd[:],
                                            op=ALU.add)

                for bp in range(NT // 4 if STAGE >= 3 else 0):
                    ph2_pair(bp)

            nc.sync.dma_start(out=out_d[:], in_=acc[:])

    nc.compile()
    return nc


def _wrap_idx(idx):
    """idx list -> [128, ceil(n/16)] int16 wrapped (idx i at [i%16, i//16]),
    replicated across the 8 groups of 16 partitions."""
    idx = np.asarray(idx, np.int16)
    n = len(idx)
    cols = -(-n // 16)
    if cols * 16 > n:
        fill = idx[-1] if n else np.int16(0)
        idx = np.concatenate([idx, np.full(cols * 16 - n, fill, np.int16)])
    w = np.ascontiguousarray(idx.reshape(cols, 16).T)
    return np.tile(w, (8, 1))


def _bank_plan(uniqs):
    """Position-based banking: slot space = ceil(max_core_uniques/128)*128
    slots split into 7 contiguous bank segments; bank b of core c takes the
    core's sorted unique ids at positions [C_b, C_b + n) (n <= P_b). Bank base
    = min over cores of the id at position C_b, so every relative index is in
    [0, 32768)."""
    smax = max(len(u) for u in uniqs)
    nbt = -(-smax // 128)
    k, r = divmod(nbt, NBANK)
    blocks = [k + 1] * r + [k] * (NBANK - r)
    P = [bl * 128 for bl in blocks]
    C = np.concatenate([[0], np.cumsum(P)])
    bases = []
    for b in range(NBANK):
        lo = min(int(u[C[b]]) for u in uniqs if C[b] < len(u))
        bases.append(lo)
        for u in uniqs:
            end = min(C[b + 1], len(u))
            if end > C[b]:
                assert int(u[end - 1]) - lo < BANK, "bank window overflow"
    return P, bases


def _host_prep(user_tensor, item_tensor, rand_index):
    it = np.clip(item_tensor.astype(np.int64) - NUM_USER, 0, NUM_ITEM - 1)
    ut = user_tensor.astype(np.int64)
    mask = np.zeros(B * G, np.float32)
    mask[np.asarray(rand_index, dtype=np.int64)] = 1.0
    mask = mask.reshape(B, G)

    cores = []
    for k in range(NCORE):
        sl = slice(k * BC, (k + 1) * BC)
        itc, utc = it[sl], ut[sl]
        cores.append((itc, utc, mask[sl], np.unique(itc), np.unique(utc)))

    PI, BI = _bank_plan([c[3] for c in cores])
    PU, BU = _bank_plan([c[4] for c in cores])
    assert sum(PI) <= BANK and sum(PU) <= BANK, (PI, PU)
    NBI4 = -(-(sum(PI) // 128) // 4) * 4
    NBU = sum(PU) // 128

    def idx_arrays(u, P, bases, nb_total):
        """Returns (l1_idx_wrapped, j_of_unique) for one core."""
        C = np.concatenate([[0], np.cumsum(P)])
        l1 = np.zeros(sum(P), np.int16)
        slots = np.zeros(len(u), np.int64)
        for b in range(NBANK):
            pos, end = int(C[b]), min(int(C[b + 1]), len(u))
            n = max(end - pos, 0)
            ids = u[pos:end]
            rel = (ids - bases[b]).astype(np.int64)
            assert n == 0 or (rel.min() >= 0 and rel.max() < BANK)
            l1[pos:pos + n] = rel.astype(np.int16)
            if n < P[b]:
                l1[pos + n:C[b + 1]] = l1[pos + n - 1] if n else 0
            slots[pos:end] = pos + np.arange(n)
        j = (slots % 128) * nb_total + slots // 128
        assert len(j) == 0 or j.max() < 32768
        return _wrap_idx(l1), j.astype(np.int16)

    per_core = []
    for (itc, utc, mkc, ui, uu) in cores:
        l1i, ji = idx_arrays(ui, PI, BI, NBI4)
        l1u, ju = idx_arrays(uu, PU, BU, NBU)
        # L2 idx in g-major order per batch tile: i = g*128 + r
        je = ji[np.searchsorted(ui, itc)]       # [2048, 17]
        jue = ju[np.searchsorted(uu, utc)]
        l2i = je.reshape(NT, 128, G).transpose(0, 2, 1).reshape(NT, EPT)
        l2u = jue.reshape(NT, 128, G).transpose(0, 2, 1).reshape(NT, EPT)

        def wrap_l2(l2):
            out = np.zeros((16, NT * ICOL), np.int16)
            for bt in range(NT):
                out[:, bt * ICOL:(bt + 1) * ICOL] = \
                    l2[bt].reshape(ICOL, 16).T
            return np.tile(out, (8, 1))

        per_core.append({
            "i1i": l1i, "i1u": l1u,
            "i2i": wrap_l2(l2i), "i2u": wrap_l2(l2u),
            "mask": np.ascontiguousarray(mkc.reshape(NT, 128, G)),
        })
    return PI, BI, PU, BU, per_core


def kernel(v_feat, id_embedding, W1, b1, W2, b2, user_tensor, item_tensor,
           rand_index):
    v_feat = np.asarray(v_feat, dtype=np.float32)
    id_embedding = np.asarray(id_embedding, dtype=np.float32)
    # evb row: [e(0:64) | v(64:192) | 0(192:256)] bf16
    evb = np.zeros((NUM_ITEM, 256), ml_dtypes.bfloat16)
    evb[:, 0:64] = id_embedding[NUM_USER:]
    evb[:, 64:192] = v_feat
    idu = np.ascontiguousarray(id_embedding[:NUM_USER])
    W1b = np.ascontiguousarray(W1, dtype=np.float32).astype(ml_dtypes.bfloat16)
    b1f = np.ascontiguousarray(b1, dtype=np.float32)
    W2f = np.ascontiguousarray(W2, dtype=np.float32)
    # pack W2 halves side by side: [128, 0:64] = W2[0:128], [128, 64:128] = W2[128:256]
    W2b = np.concatenate([W2f[0:128, :], W2f[128:256, :]], axis=1) \
        .astype(ml_dtypes.bfloat16)
    b2f = np.ascontiguousarray(b2, dtype=np.float32)

    PI, BI, PU, BU, per_core = _host_prep(user_tensor, item_tensor, rand_index)

    key = (tuple(PI), tuple(BI), tuple(PU), tuple(BU))
    if _CACHE.get("key") != key:
        _CACHE["nc"] = _build(PI, BI, PU, BU)
        _CACHE["key"] = key
    nc = _CACHE["nc"]

    in_maps = []
    for k in range(NCORE):
        m = {"evb": evb, "idu": idu, "w1": W1b, "b1": b1f, "w2": W2b,
             "b2": b2f, "ident": np.eye(128, dtype=np.float32)}
        m.update(per_core[k])
        in_maps.append(m)
    trace = bool(int(os.environ.get("KERNEL_TRACE", "0")))
    res = bass_utils.run_bass_kernel_spmd(
        nc, in_maps, core_ids=list(range(NCORE)), trace=trace)
    _CACHE["last_results"] = res
    accs = np.stack([r["acc_out"] for r in res.results])  # [8, 128, 2]
    sums = accs.sum(axis=(0, 1), dtype=np.float64)
    l1 = sums[0] / B
    l2 = sums[1] / B
    return np.array(LR_LAMBDA * l1 + (1.0 - LR_LAMBDA) * l2, dtype=np.float32)


# revision 61
# speedup vs baseline: 1.4067x; 1.0291x over previous
"""Trainium2 Bass kernel for nn_CLCRec contrastive loss (v2: dma_gather).

Strategy (fully local per core, no collectives):
  - Batch rows sharded 8 ways (2048 rows x 17 group entries per core).
  - All table row fetches use SWDGE dma_gather (0.34ns/desc issue) instead of
    per-128-row indirect DMAs (994ns fixed each); this removes the Pool-engine
    SWDGE-issue bottleneck (655us busy in v1).
  - dma_gather indices are int16 (<32768), so gathers are two-level:
      L1: the core's ~32k *unique* item/user ids, sorted (so ids are
          bank-contiguous), gathered bank-by-bank (7 banks span 200000 rows)
          into a dense relabeled slot space.
      Items are encoded (bf16 MLP), assembled into scratch rows
          [f_hat | f | e | pad] (512B) and streamed to an HBM scratch table;
          user rows go to a second f32 scratch table.
      L2: per batch-tile, ONE gather per table fetches the per-entry rows
          from scratch (slot count <= 32768 fits int16) in g-major order so
          the dst lands as [128 batch rows, 17, elems].
  - Phase 2 computes d1 = <p_hat, f_hat> (both pre-normalized) and
    d2 = <u, select(mask, f, e)>, then the standard exp/log loss tail;
    per-core partial sums are reduced on host.
"""

import os
import sys

import numpy as np

for _p in ("/opt/trn_rl_repo", os.path.expanduser("~/.axon_site/_ro/trn_rl_repo")):
    if os.path.isdir(_p) and _p not in sys.path:
        sys.path.insert(0, _p)

import ml_dtypes

import concourse.bacc as bacc
import concourse.mybir as mybir
import concourse.tile as tile
from concourse import bass_utils
from concourse.library_config import mlp as mlp_lib


F32 = mybir.dt.float32
BF16 = mybir.dt.bfloat16
I32 = mybir.dt.int32
I16 = mybir.dt.int16
AF = mybir.ActivationFunctionType
ALU = mybir.AluOpType
AX = mybir.AxisListType

NUM_USER = 200000
NUM_ITEM = 200000
DIM_E = 64
DIM_FEAT = 128
B = 16384
G = 17  # 1 + num_neg
TEMP = 0.2
LR_LAMBDA = 0.5

NCORE = 8
BC = B // NCORE          # 2048 batch rows per core
NT = BC // 128           # 16 batch tiles (128 batch rows each)
EPT = 128 * G            # 2176 entries per batch tile
ICOL = EPT // 16         # 136 idx cols per batch tile

# 7 gather banks; int16 gather indices reach < 32768 rows past each base.
BANK = 32768
NBANK = 7

_CACHE: dict = {}

_NEEDED_AF = None


def _patch_act_tables():
    """Force every activation we emit to resolve to the combined
    natural_log_exp_and_others set so no per-anchor table swapping occurs."""
    global _NEEDED_AF
    if _CACHE.get("act_patched"):
        return
    _NEEDED_AF = {AF.Ln, AF.Exp, AF.Prelu, AF.Copy, AF.Identity}
    import concourse.hw_specs as hw_specs
    orig = hw_specs.get_activation_tables

    def patched(module_arch):
        tabs = orig(module_arch)
        out = {}
        for name, fns in tabs.items():
            if name == "natural_log_exp_and_others":
                out[name] = fns
            else:
                out[name] = fns - _NEEDED_AF
        return out

    bacc.get_activation_tables = patched
    _CACHE["act_patched"] = True


def _build(PI, BI, PU, BU):
    """PI / PU: per-bank padded slot counts (multiples of 128); BI / BU: bank
    base row ids. Shared by all cores; sum(PI), sum(PU) <= 32768."""
    _patch_act_tables()
    NBI = sum(PI) // 128          # item slot blocks
    NBI4 = -(-NBI // 4) * 4       # padded to whole encoder chunks
    NBU = sum(PU) // 128          # user slot blocks
    NCH = NBI4 // 4
    assert NBI4 <= 256 and NBU <= 256

    STAGE = int(os.environ.get("KERNEL_STAGE", "3"))
    nc = bacc.Bacc("TRN2", target_bir_lowering=False, debug=False,
                   num_devices=NCORE, num_swdge_queues=4)

    evb_d = nc.dram_tensor("evb", [NUM_ITEM, 256], BF16, kind="ExternalInput")
    idu_d = nc.dram_tensor("idu", [NUM_USER, DIM_E], F32, kind="ExternalInput")
    w1_d = nc.dram_tensor("w1", [DIM_FEAT, 256], BF16, kind="ExternalInput")
    b1_d = nc.dram_tensor("b1", [256], F32, kind="ExternalInput")
    w2_d = nc.dram_tensor("w2", [128, 128], BF16, kind="ExternalInput")
    b2_d = nc.dram_tensor("b2", [DIM_E], F32, kind="ExternalInput")
    i1i_d = nc.dram_tensor("i1i", [128, sum(PI) // 16], I16, kind="ExternalInput")
    i1u_d = nc.dram_tensor("i1u", [128, sum(PU) // 16], I16, kind="ExternalInput")
    i2i_d = nc.dram_tensor("i2i", [128, NT * ICOL], I16, kind="ExternalInput")
    i2u_d = nc.dram_tensor("i2u", [128, NT * ICOL], I16, kind="ExternalInput")
    mk_d = nc.dram_tensor("mask", [NT, 128, G], F32, kind="ExternalInput")
    id_d = nc.dram_tensor("ident", [128, 128], F32, kind="ExternalInput")
    out_d = nc.dram_tensor("acc_out", [128, 2], F32, kind="ExternalOutput")

    scri_d = nc.dram_tensor("scri", [128, NBI4, 128], BF16, kind="Internal")
    scru_d = nc.dram_tensor("scru", [128, NBU, DIM_E], F32, kind="Internal")

    with tile.TileContext(nc) as tc:
        nc.gpsimd.load_library(mlp_lib)
        with tc.tile_pool(name="const", bufs=1) as cp:
            ident = cp.tile([128, 128], F32, tag="ident")
            nc.sync.dma_start(out=ident[:], in_=id_d[:])

            w1sb = cp.tile([128, 256], BF16, tag="w1sb")
            nc.sync.dma_start(out=w1sb[:], in_=w1_d[:])
            w2sb = cp.tile([128, 128], BF16, tag="w2sb")
            nc.sync.dma_start(out=w2sb[:], in_=w2_d[:])
            b1sb = cp.tile([128, 2], F32, tag="b1sb")
            nc.sync.dma_start(out=b1sb[:], in_=b1_d[:].rearrange("(h p) -> p h", p=128))
            b2sb = cp.tile([64, 1], F32, tag="b2sb")
            nc.sync.dma_start(out=b2sb[:], in_=b2_d[:].rearrange("(e o) -> e o", o=1))
            acc = cp.tile([128, 2], F32, tag="acc")
            nc.vector.memset(acc[:], 0.0)

            # single_packet gathers cap at 1024 idxs per call; round-robin the
            # 4 SWDGE queues (each runs on its own Q7 cpu pair -> parallel
            # descriptor generation).
            qrr = [0]

            def gather_split(dst3, blk0, src, idxs, coff0, total, elem):
                done = 0
                while done < total:
                    n = min(1024, total - done)
                    nc.gpsimd.dma_gather(
                        dst3[:, blk0 + done // 128:blk0 + (done + n) // 128, :],
                        src,
                        idxs[:, coff0 + done // 16:coff0 + (done + n) // 16],
                        n, n, elem, queue_num=qrr[0] % 4)
                    qrr[0] += 1
                    done += n

            # ---------------- phase I: items (gather uniques + encode) ------
            with tc.tile_pool(name="l1i", bufs=1) as ip, \
                 tc.tile_pool(name="enc", bufs=3) as ep, \
                 tc.tile_pool(name="l1u", bufs=2) as up, \
                 tc.tile_pool(name="psT", bufs=2, space="PSUM") as psT, \
                 tc.tile_pool(name="psO", bufs=2, space="PSUM") as psO, \
                 tc.tile_pool(name="psH", bufs=2, space="PSUM") as psH, \
                 tc.tile_pool(name="psF", bufs=2, space="PSUM") as psF:

                i1i = ip.tile([128, sum(PI) // 16], I16, tag="i1i")
                nc.sync.dma_start(out=i1i[:], in_=i1i_d[:])
                vdst = ip.tile([128, NBI4 * 256], BF16, tag="vdst")
                vdst3 = vdst[:].rearrange("p (b e) -> p b e", e=256)
                if NBI4 > NBI:
                    nc.vector.memset(vdst3[:, NBI:NBI4, :], 0.0)
                off = 0
                coff = 0
                for b in range(NBANK):
                    gather_split(
                        vdst3, off,
                        evb_d[BI[b]:min(BI[b] + BANK, NUM_ITEM), :],
                        i1i, coff, PI[b], 256)
                    off += PI[b] // 128
                    coff += PI[b] // 16

                # batched per-slot 1/||v||: squares+reduce in 8-block passes,
                # Ln/Exp over 32-block groups, emitted interleaved with the
                # encoder chunks so chunk 0 doesn't wait on late bank gathers
                ssall = ip.tile([128, NBI4], F32, tag="ssall")

                def ssall_group(blk0):
                    end = min(blk0 + 32, NBI4)
                    j0 = blk0
                    while j0 < end:
                        nj = min(8, end - j0)
                        sqt = ep.tile([128, 8 * 128], F32, tag="sqt", bufs=2)
                        sqt3 = sqt[:].rearrange("p (j e) -> p j e", e=DIM_FEAT)
                        nc.vector.tensor_tensor(
                            out=sqt3[:, 0:nj, :],
                            in0=vdst3[:, j0:j0 + nj, 64:192],
                            in1=vdst3[:, j0:j0 + nj, 64:192], op=ALU.mult)
                        nc.vector.tensor_reduce(
                            out=ssall[:, j0:j0 + nj], in_=sqt3[:, 0:nj, :],
                            op=ALU.add, axis=AX.X)
                        j0 += nj
                    nc.scalar.activation(out=ssall[:, blk0:end],
                                         in_=ssall[:, blk0:end], func=AF.Ln)
                    nc.scalar.activation(out=ssall[:, blk0:end],
                                         in_=ssall[:, blk0:end],
                                         func=AF.Exp, scale=-0.5)

                # user L1 runs concurrently with the encoder
                i1u = up.tile([128, sum(PU) // 16], I16, tag="i1u", bufs=1)
                nc.sync.dma_start(out=i1u[:], in_=i1u_d[:])
                # split banks into ~4 groups to keep the staging tile small
                groups = []
                cur = []
                cum = 0
                for b in range(NBANK):
                    cur.append(b)
                    cum += PU[b] // 128
                    if cum >= NBU // 6:
                        groups.append(cur)
                        cur = []
                        cum = 0
                if cur:
                    groups.append(cur)
                umax = max(sum(PU[b] // 128 for b in g) for g in groups)
                uoff = 0
                ucoff = 0
                for banks in groups:
                    blocks = sum(PU[b] // 128 for b in banks)
                    udst = up.tile([128, umax * DIM_E], F32, tag="udst", bufs=2)
                    ud3 = udst[:].rearrange("p (b e) -> p b e", e=DIM_E)
                    hoff = 0
                    for b in banks:
                        gather_split(
                            ud3, hoff,
                            idu_d[BU[b]:min(BU[b] + BANK, NUM_USER), :],
                            i1u, ucoff, PU[b], DIM_E)
                        hoff += PU[b] // 128
                        ucoff += PU[b] // 16
                    nc.sync.dma_start(
                        out=scru_d[:, uoff:uoff + blocks, :],
                        in_=ud3[:, 0:blocks, :])
                    uoff += blocks

                def enc_chunk(q):
                    v3 = vdst3[:, 4 * q:4 * q + 4, 64:192]   # [128,4,128] bf16
                    vn = ep.tile([128, 512], F32, tag="vn")
                    vn3 = vn[:].rearrange("p (j e) -> p j e", e=DIM_FEAT)
                    nc.vector.tensor_tensor(
                        out=vn3, in0=v3,
                        in1=ssall[:, 4 * q:4 * q + 4]
                            .rearrange("p (j o) -> p j o", o=1)
                            .to_broadcast([128, 4, DIM_FEAT]),
                        op=ALU.mult)
                    vT_ps = psT.tile([128, 512], F32, tag="tp")
                    for j in range(4):
                        nc.tensor.transpose(
                            out=vT_ps[:, j * 128:(j + 1) * 128],
                            in_=vn[:, j * 128:(j + 1) * 128],
                            identity=ident[:])
                    vT = ep.tile([128, 512], BF16, tag="vT")
                    nc.scalar.copy(out=vT[:], in_=vT_ps[:])
                    h_sb = ep.tile([128, 1024], BF16, tag="hsb")
                    for h in range(2):
                        h_ps = psH.tile([128, 512], F32, tag="hp")
                        nc.tensor.matmul(
                            out=h_ps[:], lhsT=w1sb[:, h * 128:(h + 1) * 128],
                            rhs=vT[:], start=True, stop=True)
                        nc.scalar.activation(
                            out=h_sb[:, h * 512:(h + 1) * 512], in_=h_ps[:],
                            func=AF.Prelu, bias=b1sb[:, h:h + 1], scale=1.0,
                            alpha=0.01)
                    f_ps = psF.tile([64, 512], F32, tag="fp")
                    nc.tensor.matmul(out=f_ps[:], lhsT=w2sb[:, 0:64],
                                     rhs=h_sb[:, 0:512], start=True, stop=False)
                    nc.tensor.matmul(out=f_ps[:], lhsT=w2sb[:, 64:128],
                                     rhs=h_sb[:, 512:1024], start=False, stop=True)
                    f_sb = ep.tile([64, 512], F32, tag="fsb")
                    nc.scalar.activation(out=f_sb[:], in_=f_ps[:], func=AF.Identity,
                                         bias=b2sb[:, 0:1])
                    Fo_ps = psO.tile([128, 256], F32, tag="fo")
                    for j in range(4):
                        nc.tensor.transpose(
                            out=Fo_ps[:, j * 64:(j + 1) * 64],
                            in_=f_sb[:, j * 128:(j + 1) * 128],
                            identity=ident[:64, :64])
                    Fo3 = Fo_ps[:].rearrange("p (j e) -> p j e", e=64)
                    sc = ep.tile([128, 4 * 128], BF16, tag="sc", bufs=3)
                    sc3 = sc[:].rearrange("p (j e) -> p j e", e=128)
                    nc.vector.tensor_copy(out=sc3[:, :, 0:64], in_=Fo3)
                    nc.scalar.activation(out=sc3[:, :, 64:128],
                                         in_=vdst3[:, 4 * q:4 * q + 4, 0:64],
                                         func=AF.Copy)
                    nc.sync.dma_start(out=scri_d[:, 4 * q:4 * q + 4, :], in_=sc3)

                for q in range(NCH if STAGE >= 1 else 0):
                    if (4 * q) % 32 == 0:
                        ssall_group(4 * q)
                    enc_chunk(q)

            # ---------------- phase III: per-entry streams + loss ----------
            with tc.tile_pool(name="ph2", bufs=3) as pp, \
                 tc.tile_pool(name="prd", bufs=3) as prp:
                i2i = pp.tile([128, NT * ICOL], I16, tag="i2i", bufs=1)
                nc.sync.dma_start(out=i2i[:], in_=i2i_d[:])
                i2u = pp.tile([128, NT * ICOL], I16, tag="i2u", bufs=1)
                nc.sync.dma_start(out=i2u[:], in_=i2u_d[:])
                rows_i = scri_d[:].rearrange("p b e -> (p b) e")
                rows_u = scru_d[:].rearrange("p b e -> (p b) e")

                def ph2_pair(bp):
                    """Gather four batch-tiles' entries per split-gather
                    (8704 idxs = 8x1024 + 512 -> 9 calls/table)."""
                    ti_p = pp.tile([128, 4 * G * 128], BF16, tag="ti", bufs=2)
                    ti_p3 = ti_p[:].rearrange("p (g e) -> p g e", e=128)
                    gather_split(ti_p3, 0, rows_i, i2i, 4 * bp * ICOL,
                                 4 * EPT, 128)
                    tu_p = pp.tile([128, 4 * G * 64], F32, tag="tu", bufs=2)
                    tu_p3 = tu_p[:].rearrange("p (g e) -> p g e", e=64)
                    gather_split(tu_p3, 0, rows_u, i2u, 4 * bp * ICOL,
                                 4 * EPT, DIM_E)
                    for half in range(4):
                        ph2_tile(4 * bp + half,
                                 ti_p[:, half * G * 128:(half + 1) * G * 128],
                                 tu_p[:, half * G * 64:(half + 1) * G * 64])

                def ph2_tile(bt, ti, tu):
                    ti3 = ti.rearrange("p (g e) -> p g e", e=128)
                    tu3 = tu.rearrange("p (g e) -> p g e", e=64)
                    mk = pp.tile([128, G], F32, tag="mk")
                    nc.sync.dma_start(out=mk[:], in_=mk_d[bt])

                    F3 = ti3[:, :, 0:64]
                    E3 = ti3[:, :, 64:128]

                    # p_hat from the positive item's raw embedding (g=0 slot)
                    E0 = ti[:, 64:128]
                    prP = pp.tile([128, 64], F32, tag="prP")
                    dPP = pp.tile([128, 1], F32, tag="dPP")
                    nc.vector.tensor_tensor(out=prP[:], in0=E0, in1=E0, op=ALU.mult)
                    nc.vector.tensor_reduce(
                        out=dPP[:], in_=prP[:].rearrange("p (o e) -> p o e", o=1),
                        op=ALU.add, axis=AX.X)
                    nc.scalar.activation(out=dPP[:], in_=dPP[:], func=AF.Ln)
                    nc.scalar.activation(out=dPP[:], in_=dPP[:], func=AF.Exp,
                                         scale=-0.5)
                    ph = pp.tile([128, 64], BF16, tag="ph")
                    nc.vector.tensor_tensor(
                        out=ph[:].rearrange("p (o e) -> p o e", o=1),
                        in0=E0.rearrange("p (o e) -> p o e", o=1),
                        in1=dPP[:].rearrange("p (o e) -> p o e", e=1)
                            .to_broadcast([128, 1, 64]),
                        op=ALU.mult)
                    ph3b = ph[:].rearrange("p (g e) -> p g e", g=1) \
                        .to_broadcast([128, G, 64])

                    # d1 = <p_hat, f> / ||f||  (f left unnormalized in scratch)
                    prA = prp.tile([128, G * 64], F32, tag="prA")
                    prA3 = prA[:].rearrange("p (g e) -> p g e", e=64)
                    dPF = pp.tile([128, G], F32, tag="dPF")
                    nc.vector.tensor_tensor(out=prA3, in0=F3, in1=ph3b, op=ALU.mult)
                    nc.vector.tensor_reduce(out=dPF[:], in_=prA3, op=ALU.add,
                                            axis=AX.X)
                    prB = prp.tile([128, G * 64], F32, tag="prB")
                    prB3 = prB[:].rearrange("p (g e) -> p g e", e=64)
                    nf2 = pp.tile([128, G], F32, tag="nf2")
                    nc.vector.tensor_tensor(out=prB3, in0=F3, in1=F3, op=ALU.mult)
                    nc.vector.tensor_reduce(out=nf2[:], in_=prB3, op=ALU.add,
                                            axis=AX.X)
                    nc.scalar.activation(out=nf2[:], in_=nf2[:], func=AF.Ln)
                    nc.scalar.activation(out=nf2[:], in_=nf2[:], func=AF.Exp,
                                         scale=-0.5)
                    d1 = pp.tile([128, G], F32, tag="d1")
                    nc.vector.tensor_tensor(out=d1[:], in0=dPF[:], in1=nf2[:],
                                            op=ALU.mult)

                    # d2 = <u, e> + m * (<u, f> - <u, e>)  (scalar select)
                    prC = prp.tile([128, G * 64], F32, tag="prC")
                    prC3 = prC[:].rearrange("p (g e) -> p g e", e=64)
                    dA = pp.tile([128, G], F32, tag="dA")
                    nc.vector.tensor_tensor(out=prC3, in0=tu3, in1=E3, op=ALU.mult)
                    nc.vector.tensor_reduce(out=dA[:], in_=prC3, op=ALU.add, axis=AX.X)
                    prD = prp.tile([128, G * 64], F32, tag="prD")
                    prD3 = prD[:].rearrange("p (g e) -> p g e", e=64)
                    dB = pp.tile([128, G], F32, tag="dB")
                    nc.vector.tensor_tensor(out=prD3, in0=tu3, in1=F3, op=ALU.mult)
                    nc.vector.tensor_reduce(out=dB[:], in_=prD3, op=ALU.add, axis=AX.X)
                    d2 = pp.tile([128, G], F32, tag="d2")
                    nc.vector.tensor_tensor(out=d2[:], in0=dB[:], in1=dA[:],
                                            op=ALU.subtract)
                    nc.vector.tensor_tensor(out=d2[:], in0=d2[:], in1=mk[:],
                                            op=ALU.mult)
                    nc.vector.tensor_tensor(out=d2[:], in0=d2[:], in1=dA[:],
                                            op=ALU.add)

                    # loss tail: -log(pos/tot) = ln(tot) - d[:,0]/T
                    s1 = pp.tile([128, G], F32, tag="s1")
                    s2 = pp.tile([128, G], F32, tag="s2")
                    tot = pp.tile([128, 2], F32, tag="tot")
                    nc.scalar.activation(out=s1[:], in_=d1[:], func=AF.Exp,
                                         scale=1.0 / TEMP, accum_out=tot[:, 0:1])
                    nc.scalar.activation(out=s2[:], in_=d2[:], func=AF.Exp,
                                         scale=1.0 / TEMP, accum_out=tot[:, 1:2])
                    nc.scalar.activation(out=tot[:], in_=tot[:], func=AF.Ln)
                    dd = pp.tile([128, 2], F32, tag="dd")
                    nc.scalar.activation(out=dd[:, 0:1], in_=d1[:, 0:1],
                                         func=AF.Copy, scale=-1.0 / TEMP)
                    nc.scalar.activation(out=dd[:, 1:2], in_=d2[:, 0:1],
                                         func=AF.Copy, scale=-1.0 / TEMP)
                    nc.vector.tensor_tensor(out=acc[:], in0=acc[:], in1=tot[:],
                                            op=ALU.add)
                    nc.vector.tensor_tensor(out=acc[:], in0=acc[:], in1=dd[:],
                                            op=ALU.add)

                for bp in range(NT // 4 if STAGE >= 3 else 0):
                    ph2_pair(bp)

            nc.sync.dma_start(out=out_d[:], in_=acc[:])

    nc.compile()
    return nc


def _wrap_idx(idx):
    """idx list -> [128, ceil(n/16)] int16 wrapped (idx i at [i%16, i//16]),
    replicated across the 8 groups of 16 partitions."""
    idx = np.asarray(idx, np.int16)
    n = len(idx)
    cols = -(-n // 16)
    if cols * 16 > n:
        fill = idx[-1] if n else np.int16(0)
        idx = np.concatenate([idx, np.full(cols * 16 - n, fill, np.int16)])
    w = np.ascontiguousarray(idx.reshape(cols, 16).T)
    return np.tile(w, (8, 1))


def _bank_plan(uniqs):
    """Position-based banking: slot space = ceil(max_core_uniques/128)*128
    slots split into 7 contiguous bank segments; bank b of core c takes the
    core's sorted unique ids at positions [C_b, C_b + n) (n <= P_b). Bank base
    = min over cores of the id at position C_b, so every relative index is in
    [0, 32768)."""
    smax = max(len(u) for u in uniqs)
    nbt = -(-smax // 128)
    k, r = divmod(nbt, NBANK)
    blocks = [k + 1] * r + [k] * (NBANK - r)
    P = [bl * 128 for bl in blocks]
    C = np.concatenate([[0], np.cumsum(P)])
    bases = []
    for b in range(NBANK):
        lo = min(int(u[C[b]]) for u in uniqs if C[b] < len(u))
        bases.append(lo)
        for u in uniqs:
            end = min(C[b + 1], len(u))
            if end > C[b]:
                assert int(u[end - 1]) - lo < BANK, "bank window overflow"
    return P, bases


def _host_prep(user_tensor, item_tensor, rand_index):
    it = np.clip(item_tensor.astype(np.int64) - NUM_USER, 0, NUM_ITEM - 1)
    ut = user_tensor.astype(np.int64)
    mask = np.zeros(B * G, np.float32)
    mask[np.asarray(rand_index, dtype=np.int64)] = 1.0
    mask = mask.reshape(B, G)

    cores = []
    for k in range(NCORE):
        sl = slice(k * BC, (k + 1) * BC)
        itc, utc = it[sl], ut[sl]
        cores.append((itc, utc, mask[sl], np.unique(itc), np.unique(utc)))

    PI, BI = _bank_plan([c[3] for c in cores])
    PU, BU = _bank_plan([c[4] for c in cores])
    assert sum(PI) <= BANK and sum(PU) <= BANK, (PI, PU)
    NBI4 = -(-(sum(PI) // 128) // 4) * 4
    NBU = sum(PU) // 128

    def idx_arrays(u, P, bases, nb_total):
        """Returns (l1_idx_wrapped, j_of_unique) for one core."""
        C = np.concatenate([[0], np.cumsum(P)])
        l1 = np.zeros(sum(P), np.int16)
        slots = np.zeros(len(u), np.int64)
        for b in range(NBANK):
            pos, end = int(C[b]), min(int(C[b + 1]), len(u))
            n = max(end - pos, 0)
            ids = u[pos:end]
            rel = (ids - bases[b]).astype(np.int64)
            assert n == 0 or (rel.min() >= 0 and rel.max() < BANK)
            l1[pos:pos + n] = rel.astype(np.int16)
            if n < P[b]:
                l1[pos + n:C[b + 1]] = l1[pos + n - 1] if n else 0
            slots[pos:end] = pos + np.arange(n)
        j = (slots % 128) * nb_total + slots // 128
        assert len(j) == 0 or j.max() < 32768
        return _wrap_idx(l1), j.astype(np.int16)

    per_core = []
    for (itc, utc, mkc, ui, uu) in cores:
        l1i, ji = idx_arrays(ui, PI, BI, NBI4)
        l1u, ju = idx_arrays(uu, PU, BU, NBU)
        # L2 idx in g-major order per batch tile: i = g*128 + r
        je = ji[np.searchsorted(ui, itc)]       # [2048, 17]
        jue = ju[np.searchsorted(uu, utc)]
        l2i = je.reshape(NT, 128, G).transpose(0, 2, 1).reshape(NT, EPT)
        l2u = jue.reshape(NT, 128, G).transpose(0, 2, 1).reshape(NT, EPT)

        def wrap_l2(l2):
            out = np.zeros((16, NT * ICOL), np.int16)
            for bt in range(NT):
                out[:, bt * ICOL:(bt + 1) * ICOL] = \
                    l2[bt].reshape(ICOL, 16).T
            return np.tile(out, (8, 1))

        per_core.append({
            "i1i": l1i, "i1u": l1u,
            "i2i": wrap_l2(l2i), "i2u": wrap_l2(l2u),
            "mask": np.ascontiguousarray(mkc.reshape(NT, 128, G)),
        })
    return PI, BI, PU, BU, per_core


def kernel(v_feat, id_embedding, W1, b1, W2, b2, user_tensor, item_tensor,
           rand_index):
    v_feat = np.asarray(v_feat, dtype=np.float32)
    id_embedding = np.asarray(id_embedding, dtype=np.float32)
    # evb row: [e(0:64) | v(64:192) | 0(192:256)] bf16
    evb = np.zeros((NUM_ITEM, 256), ml_dtypes.bfloat16)
    evb[:, 0:64] = id_embedding[NUM_USER:]
    evb[:, 64:192] = v_feat
    idu = np.ascontiguousarray(id_embedding[:NUM_USER])
    W1b = np.ascontiguousarray(W1, dtype=np.float32).astype(ml_dtypes.bfloat16)
    b1f = np.ascontiguousarray(b1, dtype=np.float32)
    W2f = np.ascontiguousarray(W2, dtype=np.float32)
    # pack W2 halves side by side: [128, 0:64] = W2[0:128], [128, 64:128] = W2[128:256]
    W2b = np.concatenate([W2f[0:128, :], W2f[128:256, :]], axis=1) \
        .astype(ml_dtypes.bfloat16)
    b2f = np.ascontiguousarray(b2, dtype=np.float32)

    PI, BI, PU, BU, per_core = _host_prep(user_tensor, item_tensor, rand_index)

    key = (tuple(PI), tuple(BI), tuple(PU), tuple(BU))
    if _CACHE.get("key") != key:
        _CACHE["nc"] = _build(PI, BI, PU, BU)
        _CACHE["key"] = key
    nc = _CACHE["nc"]

    in_maps = []
    for k in range(NCORE):
        m = {"evb": evb, "idu": idu, "w1": W1b, "b1": b1f, "w2": W2b,
             "b2": b2f, "ident": np.eye(128, dtype=np.float32)}
        m.update(per_core[k])
        in_maps.append(m)
    trace = bool(int(os.environ.get("KERNEL_TRACE", "0")))
    res = bass_utils.run_bass_kernel_spmd(
        nc, in_maps, core_ids=list(range(NCORE)), trace=trace)
    _CACHE["last_results"] = res
    accs = np.stack([r["acc_out"] for r in res.results])  # [8, 128, 2]
    sums = accs.sum(axis=(0, 1), dtype=np.float64)
    l1 = sums[0] / B
    l2 = sums[1] / B
    return np.array(LR_LAMBDA * l1 + (1.0 - LR_LAMBDA) * l2, dtype=np.float32)
